# revision 26
# baseline (speedup 1.0000x reference)
"""Trainium2 Bass kernel for nn_Joint_56487409877109 (dense transformer block).

Data-parallel over batch: 16 batches -> 2 per core x 8 cores. All activations
feature-major ("X^T": [feat_tile, 128, tokens]); every linear layer is a
natural PE matmul with no on-device transposes.

v2 design (vs v1 baseline at ~1.55ms):
  - Zero DRAM spills: MLP+proj and FFN1+FFN2 are chunk-fused (CH=256) with
    the intermediate 4096-wide activations held per-chunk in SBUF; x1/x2
    live in SBUF across phases. DMA traffic drops ~100MB -> ~48MB/core.
  - All weights for a phase are DMA'd with emission hoisted ahead of use so
    the single qSP DMA FIFO never head-of-line-blocks the PE at a boundary.
  - LayerNorm uses "broadcast stats": mean/mean-square matmuls with an
    all-ones [128,128]/D stationary produce mu/ms pre-broadcast across
    partitions in PSUM; rstd via ACT Sqrt + DVE reciprocal_approx_fast; no
    gpsimd, no [1,n] single-lane ops.
  - LN2+LN_out fused analytically: LN(LN(y)) = (y-mu)*rc with
    rc = r1/sqrt(v*r1^2 + eps), r1 = 1/sqrt(v+eps) (gains are 1, biases 0).
  - Key mask folded into the softmax Exp as a per-partition ACT bias
    (scores^T layout puts the key index on the partition axis) - no mask
    matmuls. Softmax row-sums via all-ones matmul + reciprocal_approx_fast.
  - Attention emission interleaved so LN stats (which wait on DVE chains)
    sit behind the next batch's QKV matmuls in the in-order PE queue.
  - Output f16, upcast to f32 on host.
"""

import os
import sys
import hashlib

for _p in ("/opt/trn_rl_repo", "/root/.axon_site/_ro/trn_rl_repo"):
    if os.path.isdir(_p) and _p not in sys.path:
        sys.path.append(_p)

import numpy as np
import concourse.bacc as bacc
import concourse.tile as tile
import concourse.mybir as mybir
from concourse import bass_utils, bass2jax
from concourse.bass_utils import run_bass_kernel_spmd

F16 = mybir.dt.float16
F32 = mybir.dt.float32
AF = mybir.ActivationFunctionType
OP = mybir.AluOpType

B, S, D, DH = 16, 1024, 1024, 4096
N_CORES = 8
BPC = B // N_CORES          # batches per core
T = BPC * S                 # tokens per core
KT = D // 128               # feature tiles of D
HT = DH // 128              # feature tiles of DH
CH = 256                    # token chunk for fused MLP/FFN stages
NCH = T // CH               # chunks per core (8)
ACH = 512                   # attention s-chunk (psum free dim)
EPS = 1e-5
SCALE = 1.0 / 32.0          # 1/sqrt(D), exact
MASK_BIAS = -937.5          # SCALE * -30000: exp(x-937.5) == 0 for in-range x

_CACHE_DIR = os.path.join(os.path.dirname(os.path.abspath(__file__)), ".neff_cache")


def _install_neff_cache():
    """Cache walrus NEFF output on disk keyed by BIR hash (compile is ~minutes)."""
    if getattr(bass2jax, "_neff_cache_installed", False):
        return
    orig = bass2jax.compile_bir_kernel

    def cached(bir_json, tmpdir, neff_name="file.neff"):
        try:
            os.makedirs(_CACHE_DIR, exist_ok=True)
            key = hashlib.sha256(
                bir_json if isinstance(bir_json, bytes) else bir_json.encode()
            ).hexdigest()[:32]
            path = os.path.join(_CACHE_DIR, key + ".neff")
            out_path = os.path.join(tmpdir, neff_name)
            if os.path.exists(path):
                with open(path, "rb") as f:
                    data = f.read()
                with open(out_path, "wb") as f:
                    f.write(data)
                return out_path
            res = orig(bir_json, tmpdir, neff_name)
            with open(res, "rb") as f:
                data = f.read()
            with open(path, "wb") as f:
                f.write(data)
            return res
        except Exception:
            return orig(bir_json, tmpdir, neff_name)

    bass2jax.compile_bir_kernel = cached
    bass2jax._neff_cache_installed = True


class _Emitter:
    def __init__(self, nc, tc):
        self.nc = nc
        self.tc = tc
        self._alt = 0

    def alternate(self):
        self._alt ^= 1
        return self._alt

    # ---------- broadcast-stats LayerNorm pieces ----------
    def ln_sq(self, sqp, y_aps, n, cidx, ntag=8):
        """Square each [128,n] slice; alternate ACT/DVE. Returns sq tiles."""
        nc = self.nc
        sq = []
        for k, y in enumerate(y_aps):
            t = sqp.tile([128, n], F16, tag=f"sq{k % ntag}", name=f"sq{k % ntag}")
            if (k + cidx) % 2:
                nc.scalar.activation(t[:], y, AF.Square)
            else:
                nc.vector.tensor_tensor(t[:], y, y, OP.mult)
            sq.append(t)
        return sq

    def ln_stats(self, psp, y_aps, sq_aps, n, cidx):
        """mu/ms broadcast across partitions via all-ones/D stationary."""
        nc = self.nc
        mu = psp.tile([128, n], F32, tag=f"mu{cidx % 2}", name=f"mu{cidx % 2}")
        ms = psp.tile([128, n], F32, tag=f"ms{cidx % 2}", name=f"ms{cidx % 2}")
        for k in range(KT):
            nc.tensor.matmul(mu[:], self.ones_invD[:], y_aps[k],
                             start=(k == 0), stop=(k == KT - 1))
        for k in range(KT):
            nc.tensor.matmul(ms[:], self.ones_invD[:], sq_aps[k][:],
                             start=(k == 0), stop=(k == KT - 1))
        return mu, ms

    def ln_chain(self, tp, mu, ms, n):
        """rstd16/murstd16 [128,n] from broadcast mu/ms (single LN)."""
        nc = self.nc
        musq = tp.tile([128, n], F32, tag="c_musq", name="c_musq", bufs=1)
        nc.scalar.activation(musq[:], mu[:], AF.Square)
        var = tp.tile([128, n], F32, tag="c_var", name="c_var", bufs=1)
        nc.vector.tensor_tensor(var[:], ms[:], musq[:], OP.subtract)
        std = tp.tile([128, n], F32, tag="c_std", name="c_std", bufs=1)
        nc.scalar.activation(std[:], var[:], AF.Sqrt, bias=self.epsb[:])
        rstd = tp.tile([128, n], F32, tag="c_rstd", name="c_rstd", bufs=1)
        nc.vector.reciprocal_approx_fast(rstd[:], std[:])
        rstd16 = tp.tile([128, n], F16, tag="c_rstd16", name="c_rstd16", bufs=2)
        nc.scalar.activation(rstd16[:], rstd[:], AF.Copy)
        mur16 = tp.tile([128, n], F16, tag="c_mur16", name="c_mur16", bufs=2)
        nc.vector.tensor_tensor(mur16[:], mu[:], rstd[:], OP.mult)
        return rstd16, mur16

    def ln_chain_double(self, tp, mu, ms, n):
        """Fused LN2+LN_out: rc = r1/sqrt(v*r1^2+eps), r1=1/sqrt(v+eps)."""
        nc = self.nc
        musq = tp.tile([128, n], F32, tag="c_musq", name="c_musq", bufs=1)
        nc.scalar.activation(musq[:], mu[:], AF.Square)
        var = tp.tile([128, n], F32, tag="c_var", name="c_var", bufs=1)
        nc.vector.tensor_tensor(var[:], ms[:], musq[:], OP.subtract)
        s1 = tp.tile([128, n], F32, tag="c_std", name="c_s1", bufs=1)
        nc.scalar.activation(s1[:], var[:], AF.Sqrt, bias=self.epsb[:])
        r1 = tp.tile([128, n], F32, tag="c_rstd", name="c_r1", bufs=1)
        nc.vector.reciprocal_approx_fast(r1[:], s1[:])
        r1sq = tp.tile([128, n], F32, tag="c_r1sq", name="c_r1sq", bufs=1)
        nc.vector.tensor_tensor(r1sq[:], r1[:], r1[:], OP.mult)
        w = tp.tile([128, n], F32, tag="c_musq", name="c_w", bufs=1)
        nc.vector.tensor_tensor(w[:], var[:], r1sq[:], OP.mult)
        s2 = tp.tile([128, n], F32, tag="c_std", name="c_s2", bufs=1)
        nc.scalar.activation(s2[:], w[:], AF.Sqrt, bias=self.epsb[:])
        r2 = tp.tile([128, n], F32, tag="c_r1sq", name="c_r2", bufs=1)
        nc.vector.reciprocal_approx_fast(r2[:], s2[:])
        rc = tp.tile([128, n], F32, tag="c_rc", name="c_rc", bufs=1)
        nc.vector.tensor_tensor(rc[:], r1[:], r2[:], OP.mult)
        rc16 = tp.tile([128, n], F16, tag="c_rstd16", name="c_rc16", bufs=2)
        nc.scalar.activation(rc16[:], rc[:], AF.Copy)
        mur16 = tp.tile([128, n], F16, tag="c_mur16", name="c_mur16", bufs=2)
        nc.vector.tensor_tensor(mur16[:], mu[:], rc[:], OP.mult)
        return rc16, mur16

    def ln_apply(self, ap_pool, y_aps, out_aps, rstd16, mur16, n):
        nc = self.nc
        for k in range(KT):
            t = ap_pool.tile([128, n], F16, tag=f"ap{k % 2}", name=f"ap{k % 2}", bufs=2)
            nc.vector.tensor_tensor(t[:], y_aps[k], rstd16[:], OP.mult)
            nc.vector.tensor_tensor(out_aps[k], t[:], mur16[:], OP.subtract)

    # =========================================================
    def emit(self, ins, outs):
        nc, tc = self.nc, self.tc
        from contextlib import ExitStack

        with ExitStack() as top:
            cp = top.enter_context(tc.tile_pool(name="const", bufs=1))
            self.ones_invD = cp.tile([128, 128], F16, tag="onesD", name="onesD")
            nc.vector.memset(self.ones_invD[:], 1.0 / D)
            self.ones128 = cp.tile([128, 128], F16, tag="ones128", name="ones128")
            nc.vector.memset(self.ones128[:], 1.0)
            self.epsb = cp.tile([128, 1], F32, tag="epsb", name="epsb")
            nc.vector.memset(self.epsb[:], EPS)
            self.masks = cp.tile([128, BPC * 8], F32, tag="masks", name="masks")
            nc.scalar.dma_start(self.masks[:], ins["maskb"][:])

            # x1/x2 pools are opened/closed manually at the exact emission
            # points bounding their lifetime (SBUF is tight).
            self._px1_cm = tc.tile_pool(name="px1", bufs=1)
            px1 = self._px1_cm.__enter__()
            x1 = [px1.tile([128, T], F16, tag=f"x1_{k}", name=f"x1_{k}")
                  for k in range(KT)]

            x2_sp = nc.dram_tensor("x2_spill", [KT, 128, T], F16)
            self._stage_mlp_proj(ins, x1)
            self._stage_attn(ins, x1, x2_sp)
            self._stage_ffn(ins, x2_sp, outs["outT"])

    # ---------- stage A: LN0 + MLP + proj, chunk-fused ----------
    def _stage_mlp_proj(self, ins, x1):
        nc, tc = self.nc, self.tc
        xT_d, wmlp_d, wproj_d = ins["xT"], ins["Wmlp"], ins["Wproj"]
        with (
            tc.tile_pool(name="pxc", bufs=2) as pxc,
            tc.tile_pool(name="psq", bufs=1) as psq,
            tc.tile_pool(name="plnps", bufs=1, space="PSUM") as plnps,
            tc.tile_pool(name="pchain", bufs=1) as pchain,
            tc.tile_pool(name="pxn", bufs=1) as pxn,
            tc.tile_pool(name="pwA", bufs=1) as pwA,
            tc.tile_pool(name="pwB", bufs=1) as pwB,
            tc.tile_pool(name="ph", bufs=1) as ph,
            tc.tile_pool(name="psA", bufs=4, space="PSUM") as psA,
        ):
            # weight DMAs up front (qSP FIFO: x chunks first, then weights)
            def load_xc(c, eng):
                xs = []
                for k in range(KT):
                    t = pxc.tile([128, CH], F16, tag=f"x{k}", name=f"x{k}")
                    eng.dma_start(t[:], xT_d[k][:, c * CH:(c + 1) * CH])
                    xs.append(t)
                return xs

            xcs = {0: load_xc(0, nc.scalar), 1: load_xc(1, nc.scalar)}
            wA = []
            for m in range(HT):
                wt = pwA.tile([128, KT * 128], F16, tag=f"wA{m}", name=f"wA{m}")
                nc.sync.dma_start(
                    wt[:].rearrange("p (k q) -> p k q", k=KT),
                    wmlp_d[m].rearrange("k p q -> p k q"),
                )
                wA.append(wt)
            wB = []
            for m in range(KT):
                wt = pwB.tile([128, HT * 128], F16, tag=f"wB{m}", name=f"wB{m}")
                nc.scalar.dma_start(
                    wt[:].rearrange("p (k q) -> p k q", k=HT),
                    wproj_d[m].rearrange("k p q -> p k q"),
                )
                wB.append(wt)

            stats = {}
            xns = {}

            def emit_stats(c):
                if c >= NCH:
                    return
                if c not in xcs:
                    xcs[c] = load_xc(c, nc.gpsimd)
                xc = xcs[c]
                sq = self.ln_sq(psq, [x[:] for x in xc], CH, c)
                stats[c] = self.ln_stats(plnps, [x[:] for x in xc], sq, CH, c)

            def emit_chain_apply(c):
                if c >= NCH:
                    return
                mu, ms = stats.pop(c)
                rstd16, mur16 = self.ln_chain(pchain, mu, ms, CH)
                xn = [pxn.tile([128, CH], F16, tag=f"n{k}", name=f"n{k}")
                      for k in range(KT)]
                xc = xcs.pop(c)
                self.ln_apply(pxn, [x[:] for x in xc], [x[:] for x in xn],
                              rstd16, mur16, CH)
                xns[c] = xn

            emit_stats(0)
            emit_chain_apply(0)
            emit_stats(1)
            for c in range(NCH):
                xn = xns.pop(c)
                hts = []
                for m in range(HT):
                    ps = psA.tile([128, CH], F32, tag="psA", name="psA")
                    for k in range(KT):
                        nc.tensor.matmul(ps[:], wA[m][:, k * 128:(k + 1) * 128],
                                         xn[k][:], start=(k == 0), stop=(k == KT - 1))
                    ht = ph.tile([128, CH], F16, tag=f"h{m}", name=f"h{m}")
                    if self.alternate():
                        nc.scalar.activation(ht[:], ps[:], AF.Relu)
                    else:
                        nc.vector.tensor_scalar_max(ht[:], ps[:], 0.0)
                    hts.append(ht)
                emit_chain_apply(c + 1)
                emit_stats(c + 2)
                for m2 in range(KT):
                    ps = psA.tile([128, CH], F32, tag="psA", name="psA")
                    for k2 in range(HT):
                        nc.tensor.matmul(ps[:], wB[m2][:, k2 * 128:(k2 + 1) * 128],
                                         hts[k2][:], start=(k2 == 0), stop=(k2 == HT - 1))
                    nc.vector.tensor_scalar(x1[m2][:, c * CH:(c + 1) * CH], ps[:],
                                            -100.0, 100.0, OP.max, OP.min)

    # ---------- stage B: attention ----------
    def _stage_attn(self, ins, x1, x2_sp):
        nc, tc = self.nc, self.tc
        from contextlib import ExitStack
        wq_d, wk_d, wv_d = ins["Wq"], ins["Wk"], ins["Wv"]
        SB = S // ACH  # 2

        self._pwq_cm = tc.tile_pool(name="pwq", bufs=1)
        wq_pool = self._pwq_cm.__enter__()
        with ExitStack() as stk:
            pq = stk.enter_context(tc.tile_pool(name="pq", bufs=1))
            pk = stk.enter_context(tc.tile_pool(name="pk", bufs=1))
            pv = stk.enter_context(tc.tile_pool(name="pv", bufs=1))
            pat = stk.enter_context(tc.tile_pool(name="pat", bufs=1))
            prec = stk.enter_context(tc.tile_pool(name="prec", bufs=1))
            psq = stk.enter_context(tc.tile_pool(name="psqB", bufs=1))
            plnps = stk.enter_context(tc.tile_pool(name="plnpsB", bufs=1, space="PSUM"))
            pchain = stk.enter_context(tc.tile_pool(name="pchainB", bufs=1))
            papl = stk.enter_context(tc.tile_pool(name="paplB", bufs=1))
            px2e = stk.enter_context(tc.tile_pool(name="px2e", bufs=1))
            psM = stk.enter_context(tc.tile_pool(name="psM", bufs=4, space="PSUM"))

            wq, wk, wv = [], [], []
            for m in range(KT):
                t = wq_pool.tile([128, KT * 128], F16, tag=f"wq{m}", name=f"wq{m}")
                nc.sync.dma_start(t[:].rearrange("p (k q) -> p k q", k=KT),
                                  wq_d[m].rearrange("k p q -> p k q"))
                wq.append(t)
            for m in range(KT):
                t = wq_pool.tile([128, KT * 128], F16, tag=f"wk{m}", name=f"wk{m}")
                nc.sync.dma_start(t[:].rearrange("p (k q) -> p k q", k=KT),
                                  wk_d[m].rearrange("k p q -> p k q"))
                wk.append(t)
            for k in range(KT):
                t = wq_pool.tile([128, S], F16, tag=f"wv{k}", name=f"wv{k}")
                nc.sync.dma_start(t[:], wv_d[k])
                wv.append(t)

            state = {}

            def emit_qk(b):
                qb = [pq.tile([128, S], F16, tag=f"qb{m}", name=f"qb{m}") for m in range(KT)]
                kb = [pk.tile([128, S], F16, tag=f"kb{m}", name=f"kb{m}") for m in range(KT)]
                for m in range(KT):
                    for sb in range(SB):
                        csl = slice(b * S + sb * ACH, b * S + (sb + 1) * ACH)
                        osl = slice(sb * ACH, (sb + 1) * ACH)
                        ps = psM.tile([128, ACH], F32, tag="mm", name="mm")
                        for k in range(KT):
                            nc.tensor.matmul(ps[:], wq[m][:, k * 128:(k + 1) * 128],
                                             x1[k][:, csl], start=(k == 0), stop=(k == KT - 1))
                        if self.alternate():
                            nc.scalar.activation(qb[m][:, osl], ps[:], AF.Copy)
                        else:
                            nc.vector.tensor_copy(qb[m][:, osl], ps[:])
                        ps = psM.tile([128, ACH], F32, tag="mm", name="mm")
                        for k in range(KT):
                            nc.tensor.matmul(ps[:], wk[m][:, k * 128:(k + 1) * 128],
                                             x1[k][:, csl], start=(k == 0), stop=(k == KT - 1))
                        if self.alternate():
                            nc.scalar.activation(kb[m][:, osl], ps[:], AF.Copy)
                        else:
                            nc.vector.tensor_copy(kb[m][:, osl], ps[:])
                st = state.setdefault(b, {})
                st["qb"], st["kb"] = qb, kb

            def emit_v(b):
                vb = [pv.tile([128, S], F16, tag=f"vb{t}", name=f"vb{t}") for t in range(8)]
                for t in range(8):
                    tsl = slice(b * S + t * 128, b * S + (t + 1) * 128)
                    for n in range(SB):
                        ps = psM.tile([128, ACH], F32, tag="mm", name="mm")
                        for k in range(KT):
                            nc.tensor.matmul(ps[:], x1[k][:, tsl],
                                             wv[k][:, n * ACH:(n + 1) * ACH],
                                             start=(k == 0), stop=(k == KT - 1))
                        if self.alternate():
                            nc.scalar.activation(vb[t][:, n * ACH:(n + 1) * ACH],
                                                 ps[:], AF.Copy)
                        else:
                            nc.vector.tensor_copy(vb[t][:, n * ACH:(n + 1) * ACH], ps[:])
                state.setdefault(b, {})["vb"] = vb

            def emit_scores_out(b):
                st = state[b]
                qb, kb, vb = st["qb"], st["kb"], st["vb"]
                at = [pat.tile([128, S], F16, tag=f"at{t}", name=f"at{t}") for t in range(8)]
                for t in range(8):
                    for sb in range(SB):
                        osl = slice(sb * ACH, (sb + 1) * ACH)
                        ps = psM.tile([128, ACH], F32, tag="mm", name="mm")
                        for k in range(KT):
                            nc.tensor.matmul(ps[:], kb[k][:, t * 128:(t + 1) * 128],
                                             qb[k][:, osl], start=(k == 0), stop=(k == KT - 1))
                        j = b * 8 + t
                        nc.scalar.activation(at[t][:, osl], ps[:], AF.Exp,
                                             bias=self.masks[:, j:j + 1], scale=SCALE)
                recs = []
                for sb in range(SB):
                    osl = slice(sb * ACH, (sb + 1) * ACH)
                    ps = psM.tile([128, ACH], F32, tag="mm", name="mm")
                    for t in range(8):
                        nc.tensor.matmul(ps[:], self.ones128[:], at[t][:, osl],
                                         start=(t == 0), stop=(t == 7))
                    rec = prec.tile([128, ACH], F32, tag=f"rec{sb}", name=f"rec{sb}")
                    nc.vector.reciprocal_approx_fast(rec[:], ps[:])
                    recs.append(rec)
                # attn_out reuses the dead qb tiles (freed by the scores MMs)
                aob = [pq.tile([128, S], F16, tag=f"qb{m}", name=f"ao{m}") for m in range(KT)]
                for m in range(KT):
                    for sb in range(SB):
                        osl = slice(sb * ACH, (sb + 1) * ACH)
                        ps = psM.tile([128, ACH], F32, tag="mm", name="mm")
                        for t in range(8):
                            nc.tensor.matmul(ps[:], vb[t][:, m * 128:(m + 1) * 128],
                                             at[t][:, osl], start=(t == 0), stop=(t == 7))
                        nc.vector.tensor_tensor(aob[m][:, osl], ps[:], recs[sb][:], OP.mult)
                st["aob"] = aob

            def emit_ln1(b):
                aob = state[b]["aob"]
                # y1 = x1 + attn_out, in place into aob
                for k in range(KT):
                    nc.vector.tensor_tensor(aob[k][:], x1[k][:, b * S:(b + 1) * S],
                                            aob[k][:], OP.add)
                for sb in range(SB):
                    osl = slice(sb * ACH, (sb + 1) * ACH)
                    y = [aob[k][:, osl] for k in range(KT)]
                    sq = self.ln_sq(psq, y, ACH, sb, ntag=4)
                    mu, ms = self.ln_stats(plnps, y, sq, ACH, sb)
                    rstd16, mur16 = self.ln_chain(pchain, mu, ms, ACH)
                    x2e = [px2e.tile([128, ACH], F16, tag=f"x2e{k % 4}",
                                     name=f"x2e{k % 4}", bufs=2) for k in range(KT)]
                    self.ln_apply(papl, y, [t[:] for t in x2e], rstd16, mur16, ACH)
                    for k in range(KT):
                        nc.gpsimd.dma_start(
                            x2_sp[k, :, b * S + sb * ACH: b * S + (sb + 1) * ACH],
                            x2e[k][:])

            emit_qk(0)
            emit_v(0)
            emit_scores_out(0)
            emit_v(1)
            emit_ln1(0)
            emit_qk(1)
            # Wf1[0..23] reuse the dead wq/wk/wv tile slots; their DMAs
            # release tag-by-tag as qk(1)/v(1) finish reading.
            self.wF = []
            for i in range(24):
                tag = (f"wq{i}" if i < 8 else
                       f"wk{i - 8}" if i < 16 else f"wv{i - 16}")
                wt = wq_pool.tile([128, KT * 128], F16, tag=tag, name=f"wF{i}")
                eng = nc.sync if i % 2 == 0 else nc.scalar
                eng.dma_start(wt[:].rearrange("p (k q) -> p k q", k=KT),
                              ins["Wf1"][i].rearrange("k p q -> p k q"))
                self.wF.append(wt)
            emit_scores_out(1)
            emit_ln1(1)

    def _emit_ffn_weight_loads_b(self, ins):
        """Remaining FFN weights (wF[24..31] + all of wG) spread across the
        three DMA rings in need-time order; emitted at stage-C start."""
        nc, tc = self.nc, self.tc
        self._pwFb_cm = tc.tile_pool(name="pwFb", bufs=1)
        pwFb = self._pwFb_cm.__enter__()
        wFn = {}
        for m in range(24, HT):
            wFn[m] = pwFb.tile([128, KT * 128], F16, tag=f"wF{m}", name=f"wF{m}")
        self.wG = [pwFb.tile([128, HT * 128], F16, tag=f"wG{m}", name=f"wG{m}")
                   for m in range(KT)]

        def ld_f(m, eng):
            eng.dma_start(wFn[m][:].rearrange("p (k q) -> p k q", k=KT),
                          ins["Wf1"][m].rearrange("k p q -> p k q"))

        def ld_g(m, eng):
            eng.dma_start(self.wG[m][:].rearrange("p (k q) -> p k q", k=HT),
                          ins["Wf2"][m].rearrange("k p q -> p k q"))

        # qSP ring: wF evens then wG 0,3,6 / qAct: wF odds then wG 1,4,7
        # gpsimd: wF 26,29 + wG 2,5 (after the x2 spill tail drains)
        for m in (24, 27, 30):
            ld_f(m, nc.sync)
        for m in (25, 28, 31):
            ld_f(m, nc.scalar)
        for m in (26, 29):
            ld_f(m, nc.gpsimd)
        ld_g(0, nc.sync); ld_g(1, nc.scalar); ld_g(2, nc.gpsimd)
        ld_g(3, nc.sync); ld_g(4, nc.scalar); ld_g(5, nc.gpsimd)
        ld_g(6, nc.sync); ld_g(7, nc.scalar)
        for m in range(24, HT):
            self.wF.append(wFn[m])

    # ---------- stage C: FFN1 + FFN2 + fused LN2/LN_out, chunk-fused ----------
    def _stage_ffn(self, ins, x2_sp, outT_d):
        nc, tc = self.nc, self.tc
        self._px2c_cm = tc.tile_pool(name="px2c", bufs=2)
        px2c = self._px2c_cm.__enter__()
        x2cs = {}

        def load_x2c(c, engs=None):
            if c >= NCH or c in x2cs:
                return
            xs = []
            for k in range(KT):
                t = px2c.tile([128, CH], F16, tag=f"x2c{k}", name=f"x2c{k}")
                eng = engs[k % len(engs)] if engs else nc.gpsimd
                eng.dma_start(t[:], x2_sp[k][:, c * CH:(c + 1) * CH])
                xs.append(t)
            x2cs[c] = xs

        # chunk 0 heads the two HWDGE FIFOs, ahead of the weight streams
        load_x2c(0, engs=[nc.sync, nc.scalar])
        self._emit_ffn_weight_loads_b(ins)
        load_x2c(1)
        wF, wG = self.wF, self.wG
        with (
            tc.tile_pool(name="ph2", bufs=1) as ph2,
            tc.tile_pool(name="py2", bufs=1) as py2,
            tc.tile_pool(name="psqC", bufs=1) as psq,
            tc.tile_pool(name="plnpsC", bufs=1, space="PSUM") as plnps,
            tc.tile_pool(name="pchainC", bufs=1) as pchain,
            tc.tile_pool(name="poutC", bufs=1) as pout,
            tc.tile_pool(name="psC", bufs=4, space="PSUM") as psC,
        ):
            y2s = {}

            def emit_f1(c):
                if c >= NCH:
                    return None
                x2c = x2cs[c]
                hts = []
                for m in range(HT):
                    ps = psC.tile([128, CH], F32, tag="psC", name="psC")
                    for k in range(KT):
                        nc.tensor.matmul(ps[:], wF[m][:, k * 128:(k + 1) * 128],
                                         x2c[k][:], start=(k == 0), stop=(k == KT - 1))
                    ht = ph2.tile([128, CH], F16, tag=f"h2_{m}", name=f"h2_{m}")
                    if self.alternate():
                        nc.scalar.activation(ht[:], ps[:], AF.Relu)
                    else:
                        nc.vector.tensor_scalar_max(ht[:], ps[:], 0.0)
                    hts.append(ht)
                return hts

            def emit_f2(c, hts):
                x2c = x2cs.pop(c)
                y2 = [py2.tile([128, CH], F16, tag=f"y2_{m}", name=f"y2_{m}")
                      for m in range(KT)]
                for m2 in range(KT):
                    ps = psC.tile([128, CH], F32, tag="psC", name="psC")
                    for k2 in range(HT):
                        nc.tensor.matmul(ps[:], wG[m2][:, k2 * 128:(k2 + 1) * 128],
                                         hts[k2][:], start=(k2 == 0), stop=(k2 == HT - 1))
                    nc.vector.tensor_tensor(y2[m2][:], ps[:], x2c[m2][:], OP.add)
                y2s[c] = y2

            def emit_lnout(c):
                csl = slice(c * CH, (c + 1) * CH)
                y2 = y2s.pop(c)
                y = [t[:] for t in y2]
                sq = self.ln_sq(psq, y, CH, c)
                mu, ms = self.ln_stats(plnps, y, sq, CH, c)
                rc16, mur16 = self.ln_chain_double(pchain, mu, ms, CH)
                outs = []
                for m in range(KT):
                    o = pout.tile([128, CH], F16, tag=f"o{m % 4}", name=f"o{m % 4}", bufs=2)
                    outs.append(o)
                self.ln_apply(pout, y, [o[:] for o in outs], rc16, mur16, CH)
                for m in range(KT):
                    nc.sync.dma_start(outT_d[m, :, csl], outs[m][:])

            hts = emit_f1(0)
            for c in range(NCH):
                load_x2c(c + 2)
                emit_f2(c, hts)
                hts = emit_f1(c + 1)
                emit_lnout(c)
        self._pwFb_cm.__exit__(None, None, None)
        self._px2c_cm.__exit__(None, None, None)
        self._pwq_cm.__exit__(None, None, None)
        self._px1_cm.__exit__(None, None, None)


def build_nc():
    nc = bacc.Bacc("TRN2", target_bir_lowering=False, debug=False,
                   num_devices=N_CORES)
    ins = {
        "xT": nc.dram_tensor("xT", [KT, 128, T], F16, kind="ExternalInput"),
        "maskb": nc.dram_tensor("maskb", [128, BPC * 8], F32, kind="ExternalInput"),
        "Wmlp": nc.dram_tensor("Wmlp", [HT, KT, 128, 128], F16, kind="ExternalInput"),
        "Wproj": nc.dram_tensor("Wproj", [KT, HT, 128, 128], F16, kind="ExternalInput"),
        "Wq": nc.dram_tensor("Wq", [KT, KT, 128, 128], F16, kind="ExternalInput"),
        "Wk": nc.dram_tensor("Wk", [KT, KT, 128, 128], F16, kind="ExternalInput"),
        "Wv": nc.dram_tensor("Wv", [KT, 128, D], F16, kind="ExternalInput"),
        "Wf1": nc.dram_tensor("Wf1", [HT, KT, 128, 128], F16, kind="ExternalInput"),
        "Wf2": nc.dram_tensor("Wf2", [KT, HT, 128, 128], F16, kind="ExternalInput"),
    }
    outs = {
        "outT": nc.dram_tensor("outT", [KT, 128, T], F16, kind="ExternalOutput"),
    }
    with tile.TileContext(nc) as tc:
        em = _Emitter(nc, tc)
        em.emit(ins, outs)
    nc.compile()
    return nc


def _pack_stationary(W, mt, kt):
    # [K, M] -> [M/128, K/128, 128, 128]; tile (m,k) = W[k*128:(k+1)*128, m*128:(m+1)*128]
    K, M = W.shape
    return np.ascontiguousarray(
        W.reshape(kt, 128, mt, 128).transpose(2, 0, 1, 3)
    )


def prepare_inputs(x, mask, W_mlp, W_proj, Wq, Wk, Wv, W_f1, W_f2):
    f16 = np.float16
    shared = {
        "Wmlp": _pack_stationary(W_mlp.astype(f16), HT, KT),
        "Wproj": _pack_stationary(W_proj.astype(f16), KT, HT),
        "Wq": _pack_stationary(Wq.astype(f16), KT, KT),
        "Wk": _pack_stationary(Wk.astype(f16), KT, KT),
        "Wv": np.ascontiguousarray(Wv.astype(f16).reshape(KT, 128, D)),
        "Wf1": _pack_stationary(W_f1.astype(f16), HT, KT),
        "Wf2": _pack_stationary(W_f2.astype(f16), KT, HT),
    }
    per_core = []
    for c in range(N_CORES):
        xc = x[c * BPC:(c + 1) * BPC].reshape(T, D)          # token-major
        xTc = np.ascontiguousarray(xc.T).astype(f16).reshape(KT, 128, T)
        mc = mask[c * BPC:(c + 1) * BPC]                      # [BPC, S] int32
        # [128, BPC*8] f32: column j = b*8 + t covers tokens t*128..t*128+127
        mb = np.where(mc.reshape(BPC * 8, 128).T == 0,
                      np.float32(MASK_BIAS), np.float32(0.0))
        per_core.append({"xT": xTc, "maskb": np.ascontiguousarray(mb, dtype=np.float32),
                         **shared})
    return per_core


_NC_CACHE = {}
LAST_RESULT = {}


def kernel(**inputs):
    _install_neff_cache()
    x = np.asarray(inputs["x"], dtype=np.float32)
    mask = np.asarray(inputs["mask"])
    keys = ("W_mlp", "W_proj", "Wq", "Wk", "Wv", "W_f1", "W_f2")
    ws = [np.asarray(inputs[k], dtype=np.float32) for k in keys]

    if "nc" not in _NC_CACHE:
        _NC_CACHE["nc"] = build_nc()
    nc = _NC_CACHE["nc"]

    per_core = prepare_inputs(x, mask, *ws)
    res = run_bass_kernel_spmd(nc, per_core, list(range(N_CORES)))
    LAST_RESULT["res"] = res
    out = np.empty((B, S, D), dtype=np.float32)
    for c in range(N_CORES):
        oT = res.results[c]["outT"]            # [KT, 128, T] f16
        oc = oT.reshape(D, T).T.astype(np.float32)
        out[c * BPC:(c + 1) * BPC] = oc.reshape(BPC, S, D)
    return out


# revision 28
# speedup vs baseline: 1.0052x; 1.0052x over previous
"""Trainium2 Bass kernel for nn_Joint_56487409877109 (dense transformer block).

Data-parallel over batch: 16 batches -> 2 per core x 8 cores. All activations
feature-major ("X^T": [feat_tile, 128, tokens]); every linear layer is a
natural PE matmul with no on-device transposes.

v2 design (vs v1 baseline at ~1.55ms):
  - Zero DRAM spills: MLP+proj and FFN1+FFN2 are chunk-fused (CH=256) with
    the intermediate 4096-wide activations held per-chunk in SBUF; x1/x2
    live in SBUF across phases. DMA traffic drops ~100MB -> ~48MB/core.
  - All weights for a phase are DMA'd with emission hoisted ahead of use so
    the single qSP DMA FIFO never head-of-line-blocks the PE at a boundary.
  - LayerNorm uses "broadcast stats": mean/mean-square matmuls with an
    all-ones [128,128]/D stationary produce mu/ms pre-broadcast across
    partitions in PSUM; rstd via ACT Sqrt + DVE reciprocal_approx_fast; no
    gpsimd, no [1,n] single-lane ops.
  - LN2+LN_out fused analytically: LN(LN(y)) = (y-mu)*rc with
    rc = r1/sqrt(v*r1^2 + eps), r1 = 1/sqrt(v+eps) (gains are 1, biases 0).
  - Key mask folded into the softmax Exp as a per-partition ACT bias
    (scores^T layout puts the key index on the partition axis) - no mask
    matmuls. Softmax row-sums via all-ones matmul + reciprocal_approx_fast.
  - Attention emission interleaved so LN stats (which wait on DVE chains)
    sit behind the next batch's QKV matmuls in the in-order PE queue.
  - Output f16, upcast to f32 on host.
"""

import os
import sys
import hashlib

for _p in ("/opt/trn_rl_repo", "/root/.axon_site/_ro/trn_rl_repo"):
    if os.path.isdir(_p) and _p not in sys.path:
        sys.path.append(_p)

import numpy as np
import concourse.bacc as bacc
import concourse.tile as tile
import concourse.mybir as mybir
from concourse import bass_utils, bass2jax
from concourse.bass_utils import run_bass_kernel_spmd

F16 = mybir.dt.float16
F32 = mybir.dt.float32
AF = mybir.ActivationFunctionType
OP = mybir.AluOpType

B, S, D, DH = 16, 1024, 1024, 4096
N_CORES = 8
BPC = B // N_CORES          # batches per core
T = BPC * S                 # tokens per core
KT = D // 128               # feature tiles of D
HT = DH // 128              # feature tiles of DH
CH = 256                    # token chunk for fused MLP/FFN stages
NCH = T // CH               # chunks per core (8)
ACH = 512                   # attention s-chunk (psum free dim)
EPS = 1e-5
SCALE = 1.0 / 32.0          # 1/sqrt(D), exact
MASK_BIAS = -937.5          # SCALE * -30000: exp(x-937.5) == 0 for in-range x

_CACHE_DIR = os.path.join(os.path.dirname(os.path.abspath(__file__)), ".neff_cache")


def _install_neff_cache():
    """Cache walrus NEFF output on disk keyed by BIR hash (compile is ~minutes)."""
    if getattr(bass2jax, "_neff_cache_installed", False):
        return
    orig = bass2jax.compile_bir_kernel

    def cached(bir_json, tmpdir, neff_name="file.neff"):
        try:
            os.makedirs(_CACHE_DIR, exist_ok=True)
            key = hashlib.sha256(
                bir_json if isinstance(bir_json, bytes) else bir_json.encode()
            ).hexdigest()[:32]
            path = os.path.join(_CACHE_DIR, key + ".neff")
            out_path = os.path.join(tmpdir, neff_name)
            if os.path.exists(path):
                with open(path, "rb") as f:
                    data = f.read()
                with open(out_path, "wb") as f:
                    f.write(data)
                return out_path
            res = orig(bir_json, tmpdir, neff_name)
            with open(res, "rb") as f:
                data = f.read()
            with open(path, "wb") as f:
                f.write(data)
            return res
        except Exception:
            return orig(bir_json, tmpdir, neff_name)

    bass2jax.compile_bir_kernel = cached
    bass2jax._neff_cache_installed = True


class _Emitter:
    def __init__(self, nc, tc):
        self.nc = nc
        self.tc = tc
        self._alt = 0

    def alternate(self):
        self._alt ^= 1
        return self._alt

    # ---------- broadcast-stats LayerNorm pieces ----------
    def ln_sq(self, sqp, y_aps, n, cidx, ntag=8):
        """Square each [128,n] slice; alternate ACT/DVE. Returns sq tiles."""
        nc = self.nc
        sq = []
        for k, y in enumerate(y_aps):
            t = sqp.tile([128, n], F16, tag=f"sq{k % ntag}", name=f"sq{k % ntag}")
            if (k + cidx) % 2:
                nc.scalar.activation(t[:], y, AF.Square)
            else:
                nc.vector.tensor_tensor(t[:], y, y, OP.mult)
            sq.append(t)
        return sq

    def ln_stats(self, psp, y_aps, sq_aps, n, cidx):
        """mu/ms broadcast across partitions via all-ones/D stationary."""
        nc = self.nc
        mu = psp.tile([128, n], F32, tag=f"mu{cidx % 2}", name=f"mu{cidx % 2}")
        ms = psp.tile([128, n], F32, tag=f"ms{cidx % 2}", name=f"ms{cidx % 2}")
        for k in range(KT):
            nc.tensor.matmul(mu[:], self.ones_invD[:], y_aps[k],
                             start=(k == 0), stop=(k == KT - 1))
        for k in range(KT):
            nc.tensor.matmul(ms[:], self.ones_invD[:], sq_aps[k][:],
                             start=(k == 0), stop=(k == KT - 1))
        return mu, ms

    def ln_chain(self, tp, mu, ms, n):
        """rstd16/murstd16 [128,n] from broadcast mu/ms (single LN)."""
        nc = self.nc
        musq = tp.tile([128, n], F32, tag="c_musq", name="c_musq", bufs=1)
        nc.scalar.activation(musq[:], mu[:], AF.Square)
        var = tp.tile([128, n], F32, tag="c_var", name="c_var", bufs=1)
        nc.vector.tensor_tensor(var[:], ms[:], musq[:], OP.subtract)
        std = tp.tile([128, n], F32, tag="c_std", name="c_std", bufs=1)
        nc.scalar.activation(std[:], var[:], AF.Sqrt, bias=self.epsb[:])
        rstd = tp.tile([128, n], F32, tag="c_rstd", name="c_rstd", bufs=1)
        nc.vector.reciprocal_approx_fast(rstd[:], std[:])
        rstd16 = tp.tile([128, n], F16, tag="c_rstd16", name="c_rstd16", bufs=2)
        nc.scalar.activation(rstd16[:], rstd[:], AF.Copy)
        mur16 = tp.tile([128, n], F16, tag="c_mur16", name="c_mur16", bufs=2)
        nc.vector.tensor_tensor(mur16[:], mu[:], rstd[:], OP.mult)
        return rstd16, mur16

    def ln_chain_double(self, tp, mu, ms, n):
        """Fused LN2+LN_out: rc = r1/sqrt(v*r1^2+eps), r1=1/sqrt(v+eps)."""
        nc = self.nc
        musq = tp.tile([128, n], F32, tag="c_musq", name="c_musq", bufs=1)
        nc.scalar.activation(musq[:], mu[:], AF.Square)
        var = tp.tile([128, n], F32, tag="c_var", name="c_var", bufs=1)
        nc.vector.tensor_tensor(var[:], ms[:], musq[:], OP.subtract)
        s1 = tp.tile([128, n], F32, tag="c_std", name="c_s1", bufs=1)
        nc.scalar.activation(s1[:], var[:], AF.Sqrt, bias=self.epsb[:])
        r1 = tp.tile([128, n], F32, tag="c_rstd", name="c_r1", bufs=1)
        nc.vector.reciprocal_approx_fast(r1[:], s1[:])
        r1sq = tp.tile([128, n], F32, tag="c_r1sq", name="c_r1sq", bufs=1)
        nc.vector.tensor_tensor(r1sq[:], r1[:], r1[:], OP.mult)
        w = tp.tile([128, n], F32, tag="c_musq", name="c_w", bufs=1)
        nc.vector.tensor_tensor(w[:], var[:], r1sq[:], OP.mult)
        s2 = tp.tile([128, n], F32, tag="c_std", name="c_s2", bufs=1)
        nc.scalar.activation(s2[:], w[:], AF.Sqrt, bias=self.epsb[:])
        r2 = tp.tile([128, n], F32, tag="c_r1sq", name="c_r2", bufs=1)
        nc.vector.reciprocal_approx_fast(r2[:], s2[:])
        rc = tp.tile([128, n], F32, tag="c_rc", name="c_rc", bufs=1)
        nc.vector.tensor_tensor(rc[:], r1[:], r2[:], OP.mult)
        rc16 = tp.tile([128, n], F16, tag="c_rstd16", name="c_rc16", bufs=2)
        nc.scalar.activation(rc16[:], rc[:], AF.Copy)
        mur16 = tp.tile([128, n], F16, tag="c_mur16", name="c_mur16", bufs=2)
        nc.vector.tensor_tensor(mur16[:], mu[:], rc[:], OP.mult)
        return rc16, mur16

    def ln_apply(self, ap_pool, y_aps, out_aps, rstd16, mur16, n):
        nc = self.nc
        for k in range(KT):
            t = ap_pool.tile([128, n], F16, tag=f"ap{k % 2}", name=f"ap{k % 2}", bufs=2)
            nc.vector.tensor_tensor(t[:], y_aps[k], rstd16[:], OP.mult)
            nc.vector.tensor_tensor(out_aps[k], t[:], mur16[:], OP.subtract)

    # =========================================================
    def emit(self, ins, outs):
        nc, tc = self.nc, self.tc
        from contextlib import ExitStack

        with ExitStack() as top:
            cp = top.enter_context(tc.tile_pool(name="const", bufs=1))
            self.ones_invD = cp.tile([128, 128], F16, tag="onesD", name="onesD")
            nc.vector.memset(self.ones_invD[:], 1.0 / D)
            self.ones128 = cp.tile([128, 128], F16, tag="ones128", name="ones128")
            nc.vector.memset(self.ones128[:], 1.0)
            self.epsb = cp.tile([128, 1], F32, tag="epsb", name="epsb")
            nc.vector.memset(self.epsb[:], EPS)
            self.masks = cp.tile([128, BPC * 8], F32, tag="masks", name="masks")
            nc.scalar.dma_start(self.masks[:], ins["maskb"][:])

            # x1/x2 pools are opened/closed manually at the exact emission
            # points bounding their lifetime (SBUF is tight).
            self._px1_cm = tc.tile_pool(name="px1", bufs=1)
            px1 = self._px1_cm.__enter__()
            self._px1_pool = px1
            x1 = [px1.tile([128, T], F16, tag=f"x1_{k}", name=f"x1_{k}")
                  for k in range(KT)]

            x2_sp = nc.dram_tensor("x2_spill", [KT, 128, T], F16)
            self._stage_mlp_proj(ins, x1)
            self._stage_attn(ins, x1, x2_sp)
            self._stage_ffn(ins, x2_sp, outs["outT"])

    # ---------- stage A: LN0 + MLP + proj, chunk-fused ----------
    def _stage_mlp_proj(self, ins, x1):
        nc, tc = self.nc, self.tc
        xT_d, wmlp_d, wproj_d = ins["xT"], ins["Wmlp"], ins["Wproj"]
        with (
            tc.tile_pool(name="pxc", bufs=2) as pxc,
            tc.tile_pool(name="psq", bufs=1) as psq,
            tc.tile_pool(name="plnps", bufs=1, space="PSUM") as plnps,
            tc.tile_pool(name="pchain", bufs=1) as pchain,
            tc.tile_pool(name="pxn", bufs=1) as pxn,
            tc.tile_pool(name="pwA", bufs=1) as pwA,
            tc.tile_pool(name="pwB", bufs=1) as pwB,
            tc.tile_pool(name="ph", bufs=1) as ph,
            tc.tile_pool(name="psA", bufs=4, space="PSUM") as psA,
        ):
            # weight DMAs up front (qSP FIFO: x chunks first, then weights)
            def load_xc(c, eng):
                xs = []
                for k in range(KT):
                    t = pxc.tile([128, CH], F16, tag=f"x{k}", name=f"x{k}")
                    eng.dma_start(t[:], xT_d[k][:, c * CH:(c + 1) * CH])
                    xs.append(t)
                return xs

            xcs = {0: load_xc(0, nc.scalar), 1: load_xc(1, nc.scalar)}
            wA = []
            for m in range(HT):
                wt = pwA.tile([128, KT * 128], F16, tag=f"wA{m}", name=f"wA{m}")
                eng = nc.sync if m % 2 == 0 else nc.scalar
                eng.dma_start(
                    wt[:].rearrange("p (k q) -> p k q", k=KT),
                    wmlp_d[m].rearrange("k p q -> p k q"),
                )
                wA.append(wt)
            wB = []
            for m in range(KT):
                wt = pwB.tile([128, HT * 128], F16, tag=f"wB{m}", name=f"wB{m}")
                eng = nc.sync if m % 2 == 0 else nc.scalar
                eng.dma_start(
                    wt[:].rearrange("p (k q) -> p k q", k=HT),
                    wproj_d[m].rearrange("k p q -> p k q"),
                )
                wB.append(wt)

            stats = {}
            xns = {}

            def emit_stats(c):
                if c >= NCH:
                    return
                if c not in xcs:
                    xcs[c] = load_xc(c, nc.gpsimd)
                xc = xcs[c]
                sq = self.ln_sq(psq, [x[:] for x in xc], CH, c)
                stats[c] = self.ln_stats(plnps, [x[:] for x in xc], sq, CH, c)

            def emit_chain_apply(c):
                if c >= NCH:
                    return
                mu, ms = stats.pop(c)
                rstd16, mur16 = self.ln_chain(pchain, mu, ms, CH)
                xn = [pxn.tile([128, CH], F16, tag=f"n{k}", name=f"n{k}")
                      for k in range(KT)]
                xc = xcs.pop(c)
                self.ln_apply(pxn, [x[:] for x in xc], [x[:] for x in xn],
                              rstd16, mur16, CH)
                xns[c] = xn

            emit_stats(0)
            emit_chain_apply(0)
            emit_stats(1)
            for c in range(NCH):
                xn = xns.pop(c)
                hts = []
                for m in range(HT):
                    ps = psA.tile([128, CH], F32, tag="psA", name="psA")
                    for k in range(KT):
                        nc.tensor.matmul(ps[:], wA[m][:, k * 128:(k + 1) * 128],
                                         xn[k][:], start=(k == 0), stop=(k == KT - 1))
                    ht = ph.tile([128, CH], F16, tag=f"h{m}", name=f"h{m}")
                    if self.alternate():
                        nc.scalar.activation(ht[:], ps[:], AF.Relu)
                    else:
                        nc.vector.tensor_scalar_max(ht[:], ps[:], 0.0)
                    hts.append(ht)
                emit_chain_apply(c + 1)
                emit_stats(c + 2)
                for m2 in range(KT):
                    ps = psA.tile([128, CH], F32, tag="psA", name="psA")
                    for k2 in range(HT):
                        nc.tensor.matmul(ps[:], wB[m2][:, k2 * 128:(k2 + 1) * 128],
                                         hts[k2][:], start=(k2 == 0), stop=(k2 == HT - 1))
                    nc.vector.tensor_scalar(x1[m2][:, c * CH:(c + 1) * CH], ps[:],
                                            -100.0, 100.0, OP.max, OP.min)

    # ---------- stage B: attention ----------
    def _stage_attn(self, ins, x1, x2_sp):
        nc, tc = self.nc, self.tc
        from contextlib import ExitStack
        wq_d, wk_d, wv_d = ins["Wq"], ins["Wk"], ins["Wv"]
        SB = S // ACH  # 2

        self._pwq_cm = tc.tile_pool(name="pwq", bufs=1)
        wq_pool = self._pwq_cm.__enter__()
        with ExitStack() as stk:
            pq = stk.enter_context(tc.tile_pool(name="pq", bufs=1))
            pk = stk.enter_context(tc.tile_pool(name="pk", bufs=1))
            pv = stk.enter_context(tc.tile_pool(name="pv", bufs=1))
            pat = stk.enter_context(tc.tile_pool(name="pat", bufs=1))
            prec = stk.enter_context(tc.tile_pool(name="prec", bufs=1))
            psq = stk.enter_context(tc.tile_pool(name="psqB", bufs=1))
            plnps = stk.enter_context(tc.tile_pool(name="plnpsB", bufs=1, space="PSUM"))
            pchain = stk.enter_context(tc.tile_pool(name="pchainB", bufs=1))
            papl = stk.enter_context(tc.tile_pool(name="paplB", bufs=1))
            px2e = stk.enter_context(tc.tile_pool(name="px2e", bufs=1))
            psM = stk.enter_context(tc.tile_pool(name="psM", bufs=4, space="PSUM"))

            wq, wk, wv = [], [], []
            for m in range(KT):
                t = wq_pool.tile([128, KT * 128], F16, tag=f"wq{m}", name=f"wq{m}")
                nc.sync.dma_start(t[:].rearrange("p (k q) -> p k q", k=KT),
                                  wq_d[m].rearrange("k p q -> p k q"))
                wq.append(t)
            for m in range(KT):
                t = wq_pool.tile([128, KT * 128], F16, tag=f"wk{m}", name=f"wk{m}")
                nc.sync.dma_start(t[:].rearrange("p (k q) -> p k q", k=KT),
                                  wk_d[m].rearrange("k p q -> p k q"))
                wk.append(t)
            for k in range(KT):
                t = wq_pool.tile([128, S], F16, tag=f"wv{k}", name=f"wv{k}")
                nc.sync.dma_start(t[:], wv_d[k])
                wv.append(t)

            state = {}

            def emit_qk(b):
                qb = [pq.tile([128, S], F16, tag=f"qb{m}", name=f"qb{m}") for m in range(KT)]
                kb = [pk.tile([128, S], F16, tag=f"kb{m}", name=f"kb{m}") for m in range(KT)]
                for m in range(KT):
                    for sb in range(SB):
                        csl = slice(b * S + sb * ACH, b * S + (sb + 1) * ACH)
                        osl = slice(sb * ACH, (sb + 1) * ACH)
                        ps = psM.tile([128, ACH], F32, tag="mm", name="mm")
                        for k in range(KT):
                            nc.tensor.matmul(ps[:], wq[m][:, k * 128:(k + 1) * 128],
                                             x1[k][:, csl], start=(k == 0), stop=(k == KT - 1))
                        if self.alternate():
                            nc.scalar.activation(qb[m][:, osl], ps[:], AF.Copy)
                        else:
                            nc.vector.tensor_copy(qb[m][:, osl], ps[:])
                        ps = psM.tile([128, ACH], F32, tag="mm", name="mm")
                        for k in range(KT):
                            nc.tensor.matmul(ps[:], wk[m][:, k * 128:(k + 1) * 128],
                                             x1[k][:, csl], start=(k == 0), stop=(k == KT - 1))
                        if self.alternate():
                            nc.scalar.activation(kb[m][:, osl], ps[:], AF.Copy)
                        else:
                            nc.vector.tensor_copy(kb[m][:, osl], ps[:])
                st = state.setdefault(b, {})
                st["qb"], st["kb"] = qb, kb

            def emit_v(b):
                vb = [pv.tile([128, S], F16, tag=f"vb{t}", name=f"vb{t}") for t in range(8)]
                for t in range(8):
                    tsl = slice(b * S + t * 128, b * S + (t + 1) * 128)
                    for n in range(SB):
                        ps = psM.tile([128, ACH], F32, tag="mm", name="mm")
                        for k in range(KT):
                            nc.tensor.matmul(ps[:], x1[k][:, tsl],
                                             wv[k][:, n * ACH:(n + 1) * ACH],
                                             start=(k == 0), stop=(k == KT - 1))
                        if self.alternate():
                            nc.scalar.activation(vb[t][:, n * ACH:(n + 1) * ACH],
                                                 ps[:], AF.Copy)
                        else:
                            nc.vector.tensor_copy(vb[t][:, n * ACH:(n + 1) * ACH], ps[:])
                state.setdefault(b, {})["vb"] = vb

            def emit_scores_out(b):
                st = state[b]
                qb, kb, vb = st["qb"], st["kb"], st["vb"]
                at = [pat.tile([128, S], F16, tag=f"at{t}", name=f"at{t}") for t in range(8)]
                for t in range(8):
                    for sb in range(SB):
                        osl = slice(sb * ACH, (sb + 1) * ACH)
                        ps = psM.tile([128, ACH], F32, tag="mm", name="mm")
                        for k in range(KT):
                            nc.tensor.matmul(ps[:], kb[k][:, t * 128:(t + 1) * 128],
                                             qb[k][:, osl], start=(k == 0), stop=(k == KT - 1))
                        j = b * 8 + t
                        nc.scalar.activation(at[t][:, osl], ps[:], AF.Exp,
                                             bias=self.masks[:, j:j + 1], scale=SCALE)
                recs = []
                for sb in range(SB):
                    osl = slice(sb * ACH, (sb + 1) * ACH)
                    ps = psM.tile([128, ACH], F32, tag="mm", name="mm")
                    for t in range(8):
                        nc.tensor.matmul(ps[:], self.ones128[:], at[t][:, osl],
                                         start=(t == 0), stop=(t == 7))
                    rec = prec.tile([128, ACH], F32, tag=f"rec{sb}", name=f"rec{sb}")
                    nc.vector.reciprocal_approx_fast(rec[:], ps[:])
                    recs.append(rec)
                # attn_out reuses the dead qb tiles (freed by the scores MMs)
                aob = [pq.tile([128, S], F16, tag=f"qb{m}", name=f"ao{m}") for m in range(KT)]
                for m in range(KT):
                    for sb in range(SB):
                        osl = slice(sb * ACH, (sb + 1) * ACH)
                        ps = psM.tile([128, ACH], F32, tag="mm", name="mm")
                        for t in range(8):
                            nc.tensor.matmul(ps[:], vb[t][:, m * 128:(m + 1) * 128],
                                             at[t][:, osl], start=(t == 0), stop=(t == 7))
                        nc.vector.tensor_tensor(aob[m][:, osl], ps[:], recs[sb][:], OP.mult)
                st["aob"] = aob

            def emit_ln1(b):
                aob = state[b]["aob"]
                # y1 = x1 + attn_out, in place into aob
                for k in range(KT):
                    nc.vector.tensor_tensor(aob[k][:], x1[k][:, b * S:(b + 1) * S],
                                            aob[k][:], OP.add)
                for sb in range(SB):
                    osl = slice(sb * ACH, (sb + 1) * ACH)
                    y = [aob[k][:, osl] for k in range(KT)]
                    sq = self.ln_sq(psq, y, ACH, sb, ntag=4)
                    mu, ms = self.ln_stats(plnps, y, sq, ACH, sb)
                    rstd16, mur16 = self.ln_chain(pchain, mu, ms, ACH)
                    x2e = [px2e.tile([128, ACH], F16, tag=f"x2e{k % 4}",
                                     name=f"x2e{k % 4}", bufs=2) for k in range(KT)]
                    self.ln_apply(papl, y, [t[:] for t in x2e], rstd16, mur16, ACH)
                    for k in range(KT):
                        nc.gpsimd.dma_start(
                            x2_sp[k, :, b * S + sb * ACH: b * S + (sb + 1) * ACH],
                            x2e[k][:])

            emit_qk(0)
            emit_v(0)
            emit_scores_out(0)
            emit_v(1)
            emit_ln1(0)
            emit_qk(1)
            # Wf1[0..23] reuse the dead wq/wk/wv tile slots; their DMAs
            # release tag-by-tag as qk(1)/v(1) finish reading.
            self.wF = []
            for i in range(24):
                tag = (f"wq{i}" if i < 8 else
                       f"wk{i - 8}" if i < 16 else f"wv{i - 16}")
                wt = wq_pool.tile([128, KT * 128], F16, tag=tag, name=f"wF{i}")
                eng = nc.sync if i % 2 == 0 else nc.scalar
                eng.dma_start(wt[:].rearrange("p (k q) -> p k q", k=KT),
                              ins["Wf1"][i].rearrange("k p q -> p k q"))
                self.wF.append(wt)
            emit_scores_out(1)
            emit_ln1(1)
            # Wf2 first halves (k2<16) stream into the dying x1 tiles
            self.wGa = []
            for m in range(KT):
                wt = self._px1_pool.tile([128, T], F16, tag=f"x1_{m}", name=f"wGa{m}")
                eng = nc.sync if m % 2 == 0 else nc.scalar
                eng.dma_start(wt[:].rearrange("p (k q) -> p k q", k=16),
                              ins["Wf2"][m, 0:16].rearrange("k p q -> p k q"))
                self.wGa.append(wt)

    def _emit_ffn_weight_loads_b(self, ins):
        """Remaining FFN weights (wF[24..31] + wG second halves) spread
        across the three DMA rings in need-time order at stage-C start."""
        nc, tc = self.nc, self.tc
        self._pwFb_cm = tc.tile_pool(name="pwFb", bufs=1)
        pwFb = self._pwFb_cm.__enter__()
        wFn = {}
        for m in range(24, HT):
            wFn[m] = pwFb.tile([128, KT * 128], F16, tag=f"wF{m}", name=f"wF{m}")
        self.wGb = [pwFb.tile([128, 16 * 128], F16, tag=f"wGb{m}", name=f"wGb{m}")
                    for m in range(KT)]

        def ld_f(m, eng):
            eng.dma_start(wFn[m][:].rearrange("p (k q) -> p k q", k=KT),
                          ins["Wf1"][m].rearrange("k p q -> p k q"))

        def ld_g(m, eng):
            eng.dma_start(self.wGb[m][:].rearrange("p (k q) -> p k q", k=16),
                          ins["Wf2"][m, 16:HT].rearrange("k p q -> p k q"))

        for m in (24, 27, 30):
            ld_f(m, nc.sync)
        for m in (25, 28, 31):
            ld_f(m, nc.scalar)
        for m in (26, 29):
            ld_f(m, nc.gpsimd)
        ld_g(0, nc.sync); ld_g(1, nc.scalar); ld_g(2, nc.gpsimd)
        ld_g(3, nc.sync); ld_g(4, nc.scalar); ld_g(5, nc.gpsimd)
        ld_g(6, nc.sync); ld_g(7, nc.scalar)
        for m in range(24, HT):
            self.wF.append(wFn[m])

    # ---------- stage C: FFN1 + FFN2 + fused LN2/LN_out, chunk-fused ----------
    def _stage_ffn(self, ins, x2_sp, outT_d):
        nc, tc = self.nc, self.tc
        self._px2c_cm = tc.tile_pool(name="px2c", bufs=3)
        px2c = self._px2c_cm.__enter__()
        x2cs = {}

        def load_x2c(c, engs=None):
            if c >= NCH or c in x2cs:
                return
            xs = []
            for k in range(KT):
                t = px2c.tile([128, CH], F16, tag=f"x2c{k}", name=f"x2c{k}")
                eng = engs[k % len(engs)] if engs else nc.gpsimd
                eng.dma_start(t[:], x2_sp[k][:, c * CH:(c + 1) * CH])
                xs.append(t)
            x2cs[c] = xs

        # chunk 0 heads the two HWDGE FIFOs, ahead of the weight streams
        load_x2c(0, engs=[nc.sync, nc.scalar])
        self._emit_ffn_weight_loads_b(ins)
        load_x2c(1)
        wF, wGa, wGb = self.wF, self.wGa, self.wGb
        with (
            tc.tile_pool(name="ph2", bufs=2) as ph2,
            tc.tile_pool(name="py2", bufs=1) as py2,
            tc.tile_pool(name="psqC", bufs=1) as psq,
            tc.tile_pool(name="plnpsC", bufs=1, space="PSUM") as plnps,
            tc.tile_pool(name="pchainC", bufs=1) as pchain,
            tc.tile_pool(name="poutC", bufs=1) as pout,
            tc.tile_pool(name="psC", bufs=4, space="PSUM") as psC,
        ):
            y2s = {}

            def emit_f1(c):
                if c >= NCH:
                    return None
                x2c = x2cs[c]
                hts = []
                for m in range(HT):
                    ps = psC.tile([128, CH], F32, tag="psC", name="psC")
                    for k in range(KT):
                        nc.tensor.matmul(ps[:], wF[m][:, k * 128:(k + 1) * 128],
                                         x2c[k][:], start=(k == 0), stop=(k == KT - 1))
                    ht = ph2.tile([128, CH], F16, tag=f"h2_{m}", name=f"h2_{m}")
                    if self.alternate():
                        nc.scalar.activation(ht[:], ps[:], AF.Relu)
                    else:
                        nc.vector.tensor_scalar_max(ht[:], ps[:], 0.0)
                    hts.append(ht)
                return hts

            def emit_f2(c, hts):
                x2c = x2cs.pop(c)
                y2 = [py2.tile([128, CH], F16, tag=f"y2_{m}", name=f"y2_{m}")
                      for m in range(KT)]
                for m2 in range(KT):
                    ps = psC.tile([128, CH], F32, tag="psC", name="psC")
                    for k2 in range(HT):
                        wt = wGa[m2] if k2 < 16 else wGb[m2]
                        col = (k2 % 16) * 128
                        nc.tensor.matmul(ps[:], wt[:, col:col + 128],
                                         hts[k2][:], start=(k2 == 0), stop=(k2 == HT - 1))
                    nc.vector.tensor_tensor(y2[m2][:], ps[:], x2c[m2][:], OP.add)
                y2s[c] = y2

            def emit_lnout(c):
                csl = slice(c * CH, (c + 1) * CH)
                y2 = y2s.pop(c)
                y = [t[:] for t in y2]
                sq = self.ln_sq(psq, y, CH, c)
                mu, ms = self.ln_stats(plnps, y, sq, CH, c)
                rc16, mur16 = self.ln_chain_double(pchain, mu, ms, CH)
                outs = []
                for m in range(KT):
                    o = pout.tile([128, CH], F16, tag=f"o{m % 4}", name=f"o{m % 4}", bufs=2)
                    outs.append(o)
                self.ln_apply(pout, y, [o[:] for o in outs], rc16, mur16, CH)
                for m in range(KT):
                    nc.sync.dma_start(outT_d[m, :, csl], outs[m][:])

            h_pipe = {0: emit_f1(0), 1: emit_f1(1)}
            for c in range(NCH):
                load_x2c(c + 2)
                emit_f2(c, h_pipe.pop(c))
                h_pipe[c + 2] = emit_f1(c + 2)
                emit_lnout(c)
        self._pwFb_cm.__exit__(None, None, None)
        self._px2c_cm.__exit__(None, None, None)
        self._pwq_cm.__exit__(None, None, None)
        self._px1_cm.__exit__(None, None, None)


def build_nc():
    nc = bacc.Bacc("TRN2", target_bir_lowering=False, debug=False,
                   num_devices=N_CORES)
    ins = {
        "xT": nc.dram_tensor("xT", [KT, 128, T], F16, kind="ExternalInput"),
        "maskb": nc.dram_tensor("maskb", [128, BPC * 8], F32, kind="ExternalInput"),
        "Wmlp": nc.dram_tensor("Wmlp", [HT, KT, 128, 128], F16, kind="ExternalInput"),
        "Wproj": nc.dram_tensor("Wproj", [KT, HT, 128, 128], F16, kind="ExternalInput"),
        "Wq": nc.dram_tensor("Wq", [KT, KT, 128, 128], F16, kind="ExternalInput"),
        "Wk": nc.dram_tensor("Wk", [KT, KT, 128, 128], F16, kind="ExternalInput"),
        "Wv": nc.dram_tensor("Wv", [KT, 128, D], F16, kind="ExternalInput"),
        "Wf1": nc.dram_tensor("Wf1", [HT, KT, 128, 128], F16, kind="ExternalInput"),
        "Wf2": nc.dram_tensor("Wf2", [KT, HT, 128, 128], F16, kind="ExternalInput"),
    }
    outs = {
        "outT": nc.dram_tensor("outT", [KT, 128, T], F16, kind="ExternalOutput"),
    }
    with tile.TileContext(nc) as tc:
        em = _Emitter(nc, tc)
        em.emit(ins, outs)
    nc.compile()
    return nc


def _pack_stationary(W, mt, kt):
    # [K, M] -> [M/128, K/128, 128, 128]; tile (m,k) = W[k*128:(k+1)*128, m*128:(m+1)*128]
    K, M = W.shape
    return np.ascontiguousarray(
        W.reshape(kt, 128, mt, 128).transpose(2, 0, 1, 3)
    )


def prepare_inputs(x, mask, W_mlp, W_proj, Wq, Wk, Wv, W_f1, W_f2):
    f16 = np.float16
    shared = {
        "Wmlp": _pack_stationary(W_mlp.astype(f16), HT, KT),
        "Wproj": _pack_stationary(W_proj.astype(f16), KT, HT),
        "Wq": _pack_stationary(Wq.astype(f16), KT, KT),
        "Wk": _pack_stationary(Wk.astype(f16), KT, KT),
        "Wv": np.ascontiguousarray(Wv.astype(f16).reshape(KT, 128, D)),
        "Wf1": _pack_stationary(W_f1.astype(f16), HT, KT),
        "Wf2": _pack_stationary(W_f2.astype(f16), KT, HT),
    }
    per_core = []
    for c in range(N_CORES):
        xc = x[c * BPC:(c + 1) * BPC].reshape(T, D)          # token-major
        xTc = np.ascontiguousarray(xc.T).astype(f16).reshape(KT, 128, T)
        mc = mask[c * BPC:(c + 1) * BPC]                      # [BPC, S] int32
        # [128, BPC*8] f32: column j = b*8 + t covers tokens t*128..t*128+127
        mb = np.where(mc.reshape(BPC * 8, 128).T == 0,
                      np.float32(MASK_BIAS), np.float32(0.0))
        per_core.append({"xT": xTc, "maskb": np.ascontiguousarray(mb, dtype=np.float32),
                         **shared})
    return per_core


_NC_CACHE = {}
LAST_RESULT = {}


def kernel(**inputs):
    _install_neff_cache()
    x = np.asarray(inputs["x"], dtype=np.float32)
    mask = np.asarray(inputs["mask"])
    keys = ("W_mlp", "W_proj", "Wq", "Wk", "Wv", "W_f1", "W_f2")
    ws = [np.asarray(inputs[k], dtype=np.float32) for k in keys]

    if "nc" not in _NC_CACHE:
        _NC_CACHE["nc"] = build_nc()
    nc = _NC_CACHE["nc"]

    per_core = prepare_inputs(x, mask, *ws)
    res = run_bass_kernel_spmd(nc, per_core, list(range(N_CORES)))
    LAST_RESULT["res"] = res
    out = np.empty((B, S, D), dtype=np.float32)
    for c in range(N_CORES):
        oT = res.results[c]["outT"]            # [KT, 128, T] f16
        oc = oT.reshape(D, T).T.astype(np.float32)
        out[c * BPC:(c + 1) * BPC] = oc.reshape(BPC, S, D)
    return out


# revision 29
# speedup vs baseline: 1.0565x; 1.0511x over previous
"""Trainium2 Bass kernel for nn_Joint_56487409877109 (dense transformer block).

Data-parallel over batch: 16 batches -> 2 per core x 8 cores. All activations
feature-major ("X^T": [feat_tile, 128, tokens]); every linear layer is a
natural PE matmul with no on-device transposes.

v2 design (vs v1 baseline at ~1.55ms):
  - Zero DRAM spills: MLP+proj and FFN1+FFN2 are chunk-fused (CH=256) with
    the intermediate 4096-wide activations held per-chunk in SBUF; x1/x2
    live in SBUF across phases. DMA traffic drops ~100MB -> ~48MB/core.
  - All weights for a phase are DMA'd with emission hoisted ahead of use so
    the single qSP DMA FIFO never head-of-line-blocks the PE at a boundary.
  - LayerNorm uses "broadcast stats": mean/mean-square matmuls with an
    all-ones [128,128]/D stationary produce mu/ms pre-broadcast across
    partitions in PSUM; rstd via ACT Sqrt + DVE reciprocal_approx_fast; no
    gpsimd, no [1,n] single-lane ops.
  - LN2+LN_out fused analytically: LN(LN(y)) = (y-mu)*rc with
    rc = r1/sqrt(v*r1^2 + eps), r1 = 1/sqrt(v+eps) (gains are 1, biases 0).
  - Key mask folded into the softmax Exp as a per-partition ACT bias
    (scores^T layout puts the key index on the partition axis) - no mask
    matmuls. Softmax row-sums via all-ones matmul + reciprocal_approx_fast.
  - Attention emission interleaved so LN stats (which wait on DVE chains)
    sit behind the next batch's QKV matmuls in the in-order PE queue.
  - Output f16, upcast to f32 on host.
"""

import os
import sys
import hashlib

for _p in ("/opt/trn_rl_repo", "/root/.axon_site/_ro/trn_rl_repo"):
    if os.path.isdir(_p) and _p not in sys.path:
        sys.path.append(_p)

import numpy as np
import concourse.bacc as bacc
import concourse.tile as tile
import concourse.mybir as mybir
from concourse import bass_utils, bass2jax
from concourse.bass_utils import run_bass_kernel_spmd

F16 = mybir.dt.float16
F32 = mybir.dt.float32
AF = mybir.ActivationFunctionType
OP = mybir.AluOpType

B, S, D, DH = 16, 1024, 1024, 4096
N_CORES = 8
BPC = B // N_CORES          # batches per core
T = BPC * S                 # tokens per core
KT = D // 128               # feature tiles of D
HT = DH // 128              # feature tiles of DH
CH = 256                    # token chunk for fused MLP/FFN stages
NCH = T // CH               # chunks per core (8)
ACH = 512                   # attention s-chunk (psum free dim)
EPS = 1e-5
SCALE = 1.0 / 32.0          # 1/sqrt(D), exact
MASK_BIAS = -937.5          # SCALE * -30000: exp(x-937.5) == 0 for in-range x

_CACHE_DIR = os.path.join(os.path.dirname(os.path.abspath(__file__)), ".neff_cache")


def _install_neff_cache():
    """Cache walrus NEFF output on disk keyed by BIR hash (compile is ~minutes)."""
    if getattr(bass2jax, "_neff_cache_installed", False):
        return
    orig = bass2jax.compile_bir_kernel

    def cached(bir_json, tmpdir, neff_name="file.neff"):
        try:
            os.makedirs(_CACHE_DIR, exist_ok=True)
            key = hashlib.sha256(
                bir_json if isinstance(bir_json, bytes) else bir_json.encode()
            ).hexdigest()[:32]
            path = os.path.join(_CACHE_DIR, key + ".neff")
            out_path = os.path.join(tmpdir, neff_name)
            if os.path.exists(path):
                with open(path, "rb") as f:
                    data = f.read()
                with open(out_path, "wb") as f:
                    f.write(data)
                return out_path
            res = orig(bir_json, tmpdir, neff_name)
            with open(res, "rb") as f:
                data = f.read()
            with open(path, "wb") as f:
                f.write(data)
            return res
        except Exception:
            return orig(bir_json, tmpdir, neff_name)

    bass2jax.compile_bir_kernel = cached
    bass2jax._neff_cache_installed = True


class _Emitter:
    def __init__(self, nc, tc):
        self.nc = nc
        self.tc = tc
        self._alt = 0

    def alternate(self):
        self._alt ^= 1
        return self._alt

    # ---------- broadcast-stats LayerNorm pieces ----------
    def ln_sq(self, sqp, y_aps, n, cidx, ntag=8):
        """Square each [128,n] slice; alternate ACT/DVE. Returns sq tiles."""
        nc = self.nc
        sq = []
        for k, y in enumerate(y_aps):
            t = sqp.tile([128, n], F16, tag=f"sq{k % ntag}", name=f"sq{k % ntag}")
            if (k + cidx) % 2:
                nc.scalar.activation(t[:], y, AF.Square)
            else:
                nc.vector.tensor_tensor(t[:], y, y, OP.mult)
            sq.append(t)
        return sq

    def ln_stats(self, psp, y_aps, sq_aps, n, cidx):
        """mu/ms broadcast across partitions via all-ones/D stationary."""
        nc = self.nc
        mu = psp.tile([128, n], F32, tag=f"mu{cidx % 2}", name=f"mu{cidx % 2}")
        ms = psp.tile([128, n], F32, tag=f"ms{cidx % 2}", name=f"ms{cidx % 2}")
        for k in range(KT):
            nc.tensor.matmul(mu[:], self.ones_invD[:], y_aps[k],
                             start=(k == 0), stop=(k == KT - 1))
        for k in range(KT):
            nc.tensor.matmul(ms[:], self.ones_invD[:], sq_aps[k][:],
                             start=(k == 0), stop=(k == KT - 1))
        return mu, ms

    def ln_chain(self, tp, mu, ms, n):
        """rstd16/murstd16 [128,n] from broadcast mu/ms (single LN)."""
        nc = self.nc
        musq = tp.tile([128, n], F32, tag="c_musq", name="c_musq", bufs=1)
        nc.scalar.activation(musq[:], mu[:], AF.Square)
        var = tp.tile([128, n], F32, tag="c_var", name="c_var", bufs=1)
        nc.vector.tensor_tensor(var[:], ms[:], musq[:], OP.subtract)
        std = tp.tile([128, n], F32, tag="c_std", name="c_std", bufs=1)
        nc.scalar.activation(std[:], var[:], AF.Sqrt, bias=self.epsb[:])
        rstd = tp.tile([128, n], F32, tag="c_rstd", name="c_rstd", bufs=1)
        nc.vector.reciprocal_approx_fast(rstd[:], std[:])
        rstd16 = tp.tile([128, n], F16, tag="c_rstd16", name="c_rstd16", bufs=2)
        nc.scalar.activation(rstd16[:], rstd[:], AF.Copy)
        mur16 = tp.tile([128, n], F16, tag="c_mur16", name="c_mur16", bufs=2)
        nc.vector.tensor_tensor(mur16[:], mu[:], rstd[:], OP.mult)
        return rstd16, mur16

    def ln_chain_double(self, tp, mu, ms, n):
        """Fused LN2+LN_out: rc = r1/sqrt(v*r1^2+eps), r1=1/sqrt(v+eps)."""
        nc = self.nc
        musq = tp.tile([128, n], F32, tag="c_musq", name="c_musq", bufs=1)
        nc.scalar.activation(musq[:], mu[:], AF.Square)
        var = tp.tile([128, n], F32, tag="c_var", name="c_var", bufs=1)
        nc.vector.tensor_tensor(var[:], ms[:], musq[:], OP.subtract)
        s1 = tp.tile([128, n], F32, tag="c_std", name="c_s1", bufs=1)
        nc.scalar.activation(s1[:], var[:], AF.Sqrt, bias=self.epsb[:])
        r1 = tp.tile([128, n], F32, tag="c_rstd", name="c_r1", bufs=1)
        nc.vector.reciprocal_approx_fast(r1[:], s1[:])
        r1sq = tp.tile([128, n], F32, tag="c_r1sq", name="c_r1sq", bufs=1)
        nc.vector.tensor_tensor(r1sq[:], r1[:], r1[:], OP.mult)
        w = tp.tile([128, n], F32, tag="c_musq", name="c_w", bufs=1)
        nc.vector.tensor_tensor(w[:], var[:], r1sq[:], OP.mult)
        s2 = tp.tile([128, n], F32, tag="c_std", name="c_s2", bufs=1)
        nc.scalar.activation(s2[:], w[:], AF.Sqrt, bias=self.epsb[:])
        r2 = tp.tile([128, n], F32, tag="c_r1sq", name="c_r2", bufs=1)
        nc.vector.reciprocal_approx_fast(r2[:], s2[:])
        rc = tp.tile([128, n], F32, tag="c_rc", name="c_rc", bufs=1)
        nc.vector.tensor_tensor(rc[:], r1[:], r2[:], OP.mult)
        rc16 = tp.tile([128, n], F16, tag="c_rstd16", name="c_rc16", bufs=2)
        nc.scalar.activation(rc16[:], rc[:], AF.Copy)
        mur16 = tp.tile([128, n], F16, tag="c_mur16", name="c_mur16", bufs=2)
        nc.vector.tensor_tensor(mur16[:], mu[:], rc[:], OP.mult)
        return rc16, mur16

    def ln_apply(self, ap_pool, y_aps, out_aps, rstd16, mur16, n):
        nc = self.nc
        for k in range(KT):
            t = ap_pool.tile([128, n], F16, tag=f"ap{k % 2}", name=f"ap{k % 2}", bufs=2)
            nc.vector.tensor_tensor(t[:], y_aps[k], rstd16[:], OP.mult)
            nc.vector.tensor_tensor(out_aps[k], t[:], mur16[:], OP.subtract)

    # =========================================================
    def emit(self, ins, outs):
        nc, tc = self.nc, self.tc
        from contextlib import ExitStack

        with ExitStack() as top:
            cp = top.enter_context(tc.tile_pool(name="const", bufs=1))
            self.ones_invD = cp.tile([128, 128], F16, tag="onesD", name="onesD")
            nc.vector.memset(self.ones_invD[:], 1.0 / D)
            self.ones128 = cp.tile([128, 128], F16, tag="ones128", name="ones128")
            nc.vector.memset(self.ones128[:], 1.0)
            self.epsb = cp.tile([128, 1], F32, tag="epsb", name="epsb")
            nc.vector.memset(self.epsb[:], EPS)
            self.masks = cp.tile([128, BPC * 8], F32, tag="masks", name="masks")
            nc.gpsimd.dma_start(self.masks[:], ins["maskb"][:])

            # x1/x2 pools are opened/closed manually at the exact emission
            # points bounding their lifetime (SBUF is tight).
            self._px1_cm = tc.tile_pool(name="px1", bufs=1)
            px1 = self._px1_cm.__enter__()
            self._px1_pool = px1
            x1 = [px1.tile([128, T], F16, tag=f"x1_{k}", name=f"x1_{k}")
                  for k in range(KT)]

            x2_sp = nc.dram_tensor("x2_spill", [KT, 128, T], F16)
            self._stage_mlp_proj(ins, x1)
            self._stage_attn(ins, x1, x2_sp)
            self._stage_ffn(ins, x2_sp, outs["outT"])

    # ---------- stage A: LN0 + MLP + proj, chunk-fused ----------
    def _stage_mlp_proj(self, ins, x1):
        nc, tc = self.nc, self.tc
        xT_d, wmlp_d, wproj_d = ins["xT"], ins["Wmlp"], ins["Wproj"]
        with (
            tc.tile_pool(name="pxc", bufs=2) as pxc,
            tc.tile_pool(name="psq", bufs=1) as psq,
            tc.tile_pool(name="plnps", bufs=1, space="PSUM") as plnps,
            tc.tile_pool(name="pchain", bufs=1) as pchain,
            tc.tile_pool(name="pxn", bufs=1) as pxn,
            tc.tile_pool(name="pwA", bufs=1) as pwA,
            tc.tile_pool(name="pwB", bufs=1) as pwB,
            tc.tile_pool(name="ph", bufs=1) as ph,
            tc.tile_pool(name="psA", bufs=4, space="PSUM") as psA,
        ):
            # weight DMAs up front (qSP FIFO: x chunks first, then weights)
            def load_xc(c, eng):
                xs = []
                for k in range(KT):
                    t = pxc.tile([128, CH], F16, tag=f"x{k}", name=f"x{k}")
                    eng.dma_start(t[:], xT_d[k][:, c * CH:(c + 1) * CH])
                    xs.append(t)
                return xs

            xcs = {0: load_xc(0, nc.gpsimd), 1: load_xc(1, nc.gpsimd)}
            wA = []
            for m in range(HT):
                wt = pwA.tile([128, KT * 128], F16, tag=f"wA{m}", name=f"wA{m}")
                nc.sync.dma_start(
                    wt[:].rearrange("p (k q) -> p k q", k=KT),
                    wmlp_d[m].rearrange("k p q -> p k q"),
                )
                wA.append(wt)
            wB = []
            for m in range(KT):
                wt = pwB.tile([128, HT * 128], F16, tag=f"wB{m}", name=f"wB{m}")
                nc.sync.dma_start(
                    wt[:].rearrange("p (k q) -> p k q", k=HT),
                    wproj_d[m].rearrange("k p q -> p k q"),
                )
                wB.append(wt)

            stats = {}
            xns = {}

            def emit_stats(c):
                if c >= NCH:
                    return
                if c not in xcs:
                    xcs[c] = load_xc(c, nc.gpsimd)
                xc = xcs[c]
                sq = self.ln_sq(psq, [x[:] for x in xc], CH, c)
                stats[c] = self.ln_stats(plnps, [x[:] for x in xc], sq, CH, c)

            def emit_chain_apply(c):
                if c >= NCH:
                    return
                mu, ms = stats.pop(c)
                rstd16, mur16 = self.ln_chain(pchain, mu, ms, CH)
                xn = [pxn.tile([128, CH], F16, tag=f"n{k}", name=f"n{k}")
                      for k in range(KT)]
                xc = xcs.pop(c)
                self.ln_apply(pxn, [x[:] for x in xc], [x[:] for x in xn],
                              rstd16, mur16, CH)
                xns[c] = xn

            emit_stats(0)
            emit_chain_apply(0)
            emit_stats(1)
            for c in range(NCH):
                xn = xns.pop(c)
                hts = []
                for m in range(HT):
                    ps = psA.tile([128, CH], F32, tag="psA", name="psA")
                    for k in range(KT):
                        nc.tensor.matmul(ps[:], wA[m][:, k * 128:(k + 1) * 128],
                                         xn[k][:], start=(k == 0), stop=(k == KT - 1))
                    ht = ph.tile([128, CH], F16, tag=f"h{m}", name=f"h{m}")
                    if self.alternate():
                        nc.scalar.activation(ht[:], ps[:], AF.Relu)
                    else:
                        nc.vector.tensor_scalar_max(ht[:], ps[:], 0.0)
                    hts.append(ht)
                emit_chain_apply(c + 1)
                emit_stats(c + 2)
                for m2 in range(KT):
                    ps = psA.tile([128, CH], F32, tag="psA", name="psA")
                    for k2 in range(HT):
                        nc.tensor.matmul(ps[:], wB[m2][:, k2 * 128:(k2 + 1) * 128],
                                         hts[k2][:], start=(k2 == 0), stop=(k2 == HT - 1))
                    nc.vector.tensor_scalar(x1[m2][:, c * CH:(c + 1) * CH], ps[:],
                                            -100.0, 100.0, OP.max, OP.min)

    # ---------- stage B: attention ----------
    def _stage_attn(self, ins, x1, x2_sp):
        nc, tc = self.nc, self.tc
        from contextlib import ExitStack
        wq_d, wk_d, wv_d = ins["Wq"], ins["Wk"], ins["Wv"]
        SB = S // ACH  # 2

        self._pwq_cm = tc.tile_pool(name="pwq", bufs=1)
        wq_pool = self._pwq_cm.__enter__()
        with ExitStack() as stk:
            pq = stk.enter_context(tc.tile_pool(name="pq", bufs=1))
            pk = stk.enter_context(tc.tile_pool(name="pk", bufs=1))
            pv = stk.enter_context(tc.tile_pool(name="pv", bufs=1))
            pat = stk.enter_context(tc.tile_pool(name="pat", bufs=1))
            prec = stk.enter_context(tc.tile_pool(name="prec", bufs=1))
            psq = stk.enter_context(tc.tile_pool(name="psqB", bufs=1))
            plnps = stk.enter_context(tc.tile_pool(name="plnpsB", bufs=1, space="PSUM"))
            pchain = stk.enter_context(tc.tile_pool(name="pchainB", bufs=1))
            papl = stk.enter_context(tc.tile_pool(name="paplB", bufs=1))
            px2e = stk.enter_context(tc.tile_pool(name="px2e", bufs=1))
            psM = stk.enter_context(tc.tile_pool(name="psM", bufs=4, space="PSUM"))

            wq, wk, wv = [], [], []
            for m in range(KT):
                t = wq_pool.tile([128, KT * 128], F16, tag=f"wq{m}", name=f"wq{m}")
                nc.sync.dma_start(t[:].rearrange("p (k q) -> p k q", k=KT),
                                  wq_d[m].rearrange("k p q -> p k q"))
                wq.append(t)
            for m in range(KT):
                t = wq_pool.tile([128, KT * 128], F16, tag=f"wk{m}", name=f"wk{m}")
                nc.sync.dma_start(t[:].rearrange("p (k q) -> p k q", k=KT),
                                  wk_d[m].rearrange("k p q -> p k q"))
                wk.append(t)
            for k in range(KT):
                t = wq_pool.tile([128, S], F16, tag=f"wv{k}", name=f"wv{k}")
                nc.sync.dma_start(t[:], wv_d[k])
                wv.append(t)

            state = {}

            def emit_qk(b):
                qb = [pq.tile([128, S], F16, tag=f"qb{m}", name=f"qb{m}") for m in range(KT)]
                kb = [pk.tile([128, S], F16, tag=f"kb{m}", name=f"kb{m}") for m in range(KT)]
                for m in range(KT):
                    for sb in range(SB):
                        csl = slice(b * S + sb * ACH, b * S + (sb + 1) * ACH)
                        osl = slice(sb * ACH, (sb + 1) * ACH)
                        ps = psM.tile([128, ACH], F32, tag="mm", name="mm")
                        for k in range(KT):
                            nc.tensor.matmul(ps[:], wq[m][:, k * 128:(k + 1) * 128],
                                             x1[k][:, csl], start=(k == 0), stop=(k == KT - 1))
                        if self.alternate():
                            nc.scalar.activation(qb[m][:, osl], ps[:], AF.Copy)
                        else:
                            nc.vector.tensor_copy(qb[m][:, osl], ps[:])
                        ps = psM.tile([128, ACH], F32, tag="mm", name="mm")
                        for k in range(KT):
                            nc.tensor.matmul(ps[:], wk[m][:, k * 128:(k + 1) * 128],
                                             x1[k][:, csl], start=(k == 0), stop=(k == KT - 1))
                        if self.alternate():
                            nc.scalar.activation(kb[m][:, osl], ps[:], AF.Copy)
                        else:
                            nc.vector.tensor_copy(kb[m][:, osl], ps[:])
                st = state.setdefault(b, {})
                st["qb"], st["kb"] = qb, kb

            def emit_v(b):
                vb = [pv.tile([128, S], F16, tag=f"vb{t}", name=f"vb{t}") for t in range(8)]
                for t in range(8):
                    tsl = slice(b * S + t * 128, b * S + (t + 1) * 128)
                    for n in range(SB):
                        ps = psM.tile([128, ACH], F32, tag="mm", name="mm")
                        for k in range(KT):
                            nc.tensor.matmul(ps[:], x1[k][:, tsl],
                                             wv[k][:, n * ACH:(n + 1) * ACH],
                                             start=(k == 0), stop=(k == KT - 1))
                        if self.alternate():
                            nc.scalar.activation(vb[t][:, n * ACH:(n + 1) * ACH],
                                                 ps[:], AF.Copy)
                        else:
                            nc.vector.tensor_copy(vb[t][:, n * ACH:(n + 1) * ACH], ps[:])
                state.setdefault(b, {})["vb"] = vb

            def emit_scores_out(b):
                st = state[b]
                qb, kb, vb = st["qb"], st["kb"], st["vb"]
                at = [pat.tile([128, S], F16, tag=f"at{t}", name=f"at{t}") for t in range(8)]
                for t in range(8):
                    for sb in range(SB):
                        osl = slice(sb * ACH, (sb + 1) * ACH)
                        ps = psM.tile([128, ACH], F32, tag="mm", name="mm")
                        for k in range(KT):
                            nc.tensor.matmul(ps[:], kb[k][:, t * 128:(t + 1) * 128],
                                             qb[k][:, osl], start=(k == 0), stop=(k == KT - 1))
                        j = b * 8 + t
                        nc.scalar.activation(at[t][:, osl], ps[:], AF.Exp,
                                             bias=self.masks[:, j:j + 1], scale=SCALE)
                recs = []
                for sb in range(SB):
                    osl = slice(sb * ACH, (sb + 1) * ACH)
                    ps = psM.tile([128, ACH], F32, tag="mm", name="mm")
                    for t in range(8):
                        nc.tensor.matmul(ps[:], self.ones128[:], at[t][:, osl],
                                         start=(t == 0), stop=(t == 7))
                    rec = prec.tile([128, ACH], F32, tag=f"rec{sb}", name=f"rec{sb}")
                    nc.vector.reciprocal_approx_fast(rec[:], ps[:])
                    recs.append(rec)
                # attn_out reuses the dead qb tiles (freed by the scores MMs)
                aob = [pq.tile([128, S], F16, tag=f"qb{m}", name=f"ao{m}") for m in range(KT)]
                for m in range(KT):
                    for sb in range(SB):
                        osl = slice(sb * ACH, (sb + 1) * ACH)
                        ps = psM.tile([128, ACH], F32, tag="mm", name="mm")
                        for t in range(8):
                            nc.tensor.matmul(ps[:], vb[t][:, m * 128:(m + 1) * 128],
                                             at[t][:, osl], start=(t == 0), stop=(t == 7))
                        nc.vector.tensor_tensor(aob[m][:, osl], ps[:], recs[sb][:], OP.mult)
                st["aob"] = aob

            def emit_ln1(b):
                aob = state[b]["aob"]
                # y1 = x1 + attn_out, in place into aob
                for k in range(KT):
                    nc.vector.tensor_tensor(aob[k][:], x1[k][:, b * S:(b + 1) * S],
                                            aob[k][:], OP.add)
                for sb in range(SB):
                    osl = slice(sb * ACH, (sb + 1) * ACH)
                    y = [aob[k][:, osl] for k in range(KT)]
                    sq = self.ln_sq(psq, y, ACH, sb, ntag=4)
                    mu, ms = self.ln_stats(plnps, y, sq, ACH, sb)
                    rstd16, mur16 = self.ln_chain(pchain, mu, ms, ACH)
                    x2e = [px2e.tile([128, ACH], F16, tag=f"x2e{k % 4}",
                                     name=f"x2e{k % 4}", bufs=2) for k in range(KT)]
                    self.ln_apply(papl, y, [t[:] for t in x2e], rstd16, mur16, ACH)
                    for k in range(KT):
                        nc.gpsimd.dma_start(
                            x2_sp[k, :, b * S + sb * ACH: b * S + (sb + 1) * ACH],
                            x2e[k][:])

            emit_qk(0)
            emit_v(0)
            emit_scores_out(0)
            emit_v(1)
            emit_ln1(0)
            emit_qk(1)
            # Wf1[0..23] reuse the dead wq/wk/wv tile slots; their DMAs
            # release tag-by-tag as qk(1)/v(1) finish reading.
            self.wF = []
            for i in range(24):
                tag = (f"wq{i}" if i < 8 else
                       f"wk{i - 8}" if i < 16 else f"wv{i - 16}")
                wt = wq_pool.tile([128, KT * 128], F16, tag=tag, name=f"wF{i}")
                nc.sync.dma_start(wt[:].rearrange("p (k q) -> p k q", k=KT),
                                  ins["Wf1"][i].rearrange("k p q -> p k q"))
                self.wF.append(wt)
            emit_scores_out(1)
            emit_ln1(1)
            # Wf2 first halves (k2<16) stream into the dying x1 tiles
            self.wGa = []
            for m in range(KT):
                wt = self._px1_pool.tile([128, T], F16, tag=f"x1_{m}", name=f"wGa{m}")
                nc.sync.dma_start(wt[:].rearrange("p (k q) -> p k q", k=16),
                                  ins["Wf2"][m, 0:16].rearrange("k p q -> p k q"))
                self.wGa.append(wt)

    def _emit_ffn_weight_loads_b(self, ins):
        """Remaining FFN weights (wF[24..31] + wG second halves) spread
        across the three DMA rings in need-time order at stage-C start."""
        nc, tc = self.nc, self.tc
        self._pwFb_cm = tc.tile_pool(name="pwFb", bufs=1)
        pwFb = self._pwFb_cm.__enter__()
        wFn = {}
        for m in range(24, HT):
            wFn[m] = pwFb.tile([128, KT * 128], F16, tag=f"wF{m}", name=f"wF{m}")
        self.wGb = [pwFb.tile([128, 16 * 128], F16, tag=f"wGb{m}", name=f"wGb{m}")
                    for m in range(KT)]

        def ld_f(m, eng):
            eng.dma_start(wFn[m][:].rearrange("p (k q) -> p k q", k=KT),
                          ins["Wf1"][m].rearrange("k p q -> p k q"))

        def ld_g(m, eng):
            eng.dma_start(self.wGb[m][:].rearrange("p (k q) -> p k q", k=16),
                          ins["Wf2"][m, 16:HT].rearrange("k p q -> p k q"))

        for m in range(24, HT):
            ld_f(m, nc.sync)
        for m in range(KT):
            ld_g(m, nc.sync)
        for m in range(24, HT):
            self.wF.append(wFn[m])

    # ---------- stage C: FFN1 + FFN2 + fused LN2/LN_out, chunk-fused ----------
    def _stage_ffn(self, ins, x2_sp, outT_d):
        nc, tc = self.nc, self.tc
        self._px2c_cm = tc.tile_pool(name="px2c", bufs=3)
        px2c = self._px2c_cm.__enter__()
        x2cs = {}

        def load_x2c(c, engs=None):
            if c >= NCH or c in x2cs:
                return
            xs = []
            for k in range(KT):
                t = px2c.tile([128, CH], F16, tag=f"x2c{k}", name=f"x2c{k}")
                eng = engs[k % len(engs)] if engs else nc.gpsimd
                eng.dma_start(t[:], x2_sp[k][:, c * CH:(c + 1) * CH])
                xs.append(t)
            x2cs[c] = xs

        # chunk 0 heads the qSP FIFO, ahead of the weight streams
        load_x2c(0, engs=[nc.sync])
        self._emit_ffn_weight_loads_b(ins)
        load_x2c(1)
        wF, wGa, wGb = self.wF, self.wGa, self.wGb
        with (
            tc.tile_pool(name="ph2", bufs=2) as ph2,
            tc.tile_pool(name="py2", bufs=1) as py2,
            tc.tile_pool(name="psqC", bufs=1) as psq,
            tc.tile_pool(name="plnpsC", bufs=1, space="PSUM") as plnps,
            tc.tile_pool(name="pchainC", bufs=1) as pchain,
            tc.tile_pool(name="poutC", bufs=1) as pout,
            tc.tile_pool(name="psC", bufs=4, space="PSUM") as psC,
        ):
            y2s = {}

            def emit_f1(c):
                if c >= NCH:
                    return None
                x2c = x2cs[c]
                hts = []
                for m in range(HT):
                    ps = psC.tile([128, CH], F32, tag="psC", name="psC")
                    for k in range(KT):
                        nc.tensor.matmul(ps[:], wF[m][:, k * 128:(k + 1) * 128],
                                         x2c[k][:], start=(k == 0), stop=(k == KT - 1))
                    ht = ph2.tile([128, CH], F16, tag=f"h2_{m}", name=f"h2_{m}")
                    if self.alternate():
                        nc.scalar.activation(ht[:], ps[:], AF.Relu)
                    else:
                        nc.vector.tensor_scalar_max(ht[:], ps[:], 0.0)
                    hts.append(ht)
                return hts

            def emit_f2(c, hts):
                x2c = x2cs.pop(c)
                y2 = [py2.tile([128, CH], F16, tag=f"y2_{m}", name=f"y2_{m}")
                      for m in range(KT)]
                for m2 in range(KT):
                    ps = psC.tile([128, CH], F32, tag="psC", name="psC")
                    for k2 in range(HT):
                        wt = wGa[m2] if k2 < 16 else wGb[m2]
                        col = (k2 % 16) * 128
                        nc.tensor.matmul(ps[:], wt[:, col:col + 128],
                                         hts[k2][:], start=(k2 == 0), stop=(k2 == HT - 1))
                    nc.vector.tensor_tensor(y2[m2][:], ps[:], x2c[m2][:], OP.add)
                y2s[c] = y2

            def emit_lnout(c):
                csl = slice(c * CH, (c + 1) * CH)
                y2 = y2s.pop(c)
                y = [t[:] for t in y2]
                sq = self.ln_sq(psq, y, CH, c)
                mu, ms = self.ln_stats(plnps, y, sq, CH, c)
                rc16, mur16 = self.ln_chain_double(pchain, mu, ms, CH)
                outs = []
                for m in range(KT):
                    o = pout.tile([128, CH], F16, tag=f"o{m % 4}", name=f"o{m % 4}", bufs=2)
                    outs.append(o)
                self.ln_apply(pout, y, [o[:] for o in outs], rc16, mur16, CH)
                for m in range(KT):
                    nc.sync.dma_start(outT_d[m, :, csl], outs[m][:])

            h_pipe = {0: emit_f1(0), 1: emit_f1(1)}
            for c in range(NCH):
                load_x2c(c + 2)
                emit_f2(c, h_pipe.pop(c))
                h_pipe[c + 2] = emit_f1(c + 2)
                emit_lnout(c)
        self._pwFb_cm.__exit__(None, None, None)
        self._px2c_cm.__exit__(None, None, None)
        self._pwq_cm.__exit__(None, None, None)
        self._px1_cm.__exit__(None, None, None)


def build_nc():
    nc = bacc.Bacc("TRN2", target_bir_lowering=False, debug=False,
                   num_devices=N_CORES)
    ins = {
        "xT": nc.dram_tensor("xT", [KT, 128, T], F16, kind="ExternalInput"),
        "maskb": nc.dram_tensor("maskb", [128, BPC * 8], F32, kind="ExternalInput"),
        "Wmlp": nc.dram_tensor("Wmlp", [HT, KT, 128, 128], F16, kind="ExternalInput"),
        "Wproj": nc.dram_tensor("Wproj", [KT, HT, 128, 128], F16, kind="ExternalInput"),
        "Wq": nc.dram_tensor("Wq", [KT, KT, 128, 128], F16, kind="ExternalInput"),
        "Wk": nc.dram_tensor("Wk", [KT, KT, 128, 128], F16, kind="ExternalInput"),
        "Wv": nc.dram_tensor("Wv", [KT, 128, D], F16, kind="ExternalInput"),
        "Wf1": nc.dram_tensor("Wf1", [HT, KT, 128, 128], F16, kind="ExternalInput"),
        "Wf2": nc.dram_tensor("Wf2", [KT, HT, 128, 128], F16, kind="ExternalInput"),
    }
    outs = {
        "outT": nc.dram_tensor("outT", [KT, 128, T], F16, kind="ExternalOutput"),
    }
    with tile.TileContext(nc) as tc:
        em = _Emitter(nc, tc)
        em.emit(ins, outs)
    nc.compile()
    return nc


def _pack_stationary(W, mt, kt):
    # [K, M] -> [M/128, K/128, 128, 128]; tile (m,k) = W[k*128:(k+1)*128, m*128:(m+1)*128]
    K, M = W.shape
    return np.ascontiguousarray(
        W.reshape(kt, 128, mt, 128).transpose(2, 0, 1, 3)
    )


def prepare_inputs(x, mask, W_mlp, W_proj, Wq, Wk, Wv, W_f1, W_f2):
    f16 = np.float16
    shared = {
        "Wmlp": _pack_stationary(W_mlp.astype(f16), HT, KT),
        "Wproj": _pack_stationary(W_proj.astype(f16), KT, HT),
        "Wq": _pack_stationary(Wq.astype(f16), KT, KT),
        "Wk": _pack_stationary(Wk.astype(f16), KT, KT),
        "Wv": np.ascontiguousarray(Wv.astype(f16).reshape(KT, 128, D)),
        "Wf1": _pack_stationary(W_f1.astype(f16), HT, KT),
        "Wf2": _pack_stationary(W_f2.astype(f16), KT, HT),
    }
    per_core = []
    for c in range(N_CORES):
        xc = x[c * BPC:(c + 1) * BPC].reshape(T, D)          # token-major
        xTc = np.ascontiguousarray(xc.T).astype(f16).reshape(KT, 128, T)
        mc = mask[c * BPC:(c + 1) * BPC]                      # [BPC, S] int32
        # [128, BPC*8] f32: column j = b*8 + t covers tokens t*128..t*128+127
        mb = np.where(mc.reshape(BPC * 8, 128).T == 0,
                      np.float32(MASK_BIAS), np.float32(0.0))
        per_core.append({"xT": xTc, "maskb": np.ascontiguousarray(mb, dtype=np.float32),
                         **shared})
    return per_core


_NC_CACHE = {}
LAST_RESULT = {}


def kernel(**inputs):
    _install_neff_cache()
    x = np.asarray(inputs["x"], dtype=np.float32)
    mask = np.asarray(inputs["mask"])
    keys = ("W_mlp", "W_proj", "Wq", "Wk", "Wv", "W_f1", "W_f2")
    ws = [np.asarray(inputs[k], dtype=np.float32) for k in keys]

    if "nc" not in _NC_CACHE:
        _NC_CACHE["nc"] = build_nc()
    nc = _NC_CACHE["nc"]

    per_core = prepare_inputs(x, mask, *ws)
    res = run_bass_kernel_spmd(nc, per_core, list(range(N_CORES)))
    LAST_RESULT["res"] = res
    out = np.empty((B, S, D), dtype=np.float32)
    for c in range(N_CORES):
        oT = res.results[c]["outT"]            # [KT, 128, T] f16
        oc = oT.reshape(D, T).T.astype(np.float32)
        out[c * BPC:(c + 1) * BPC] = oc.reshape(BPC, S, D)
    return out
